# revision 20
# baseline (speedup 1.0000x reference)
"""DualPathAttention Trainium2 Bass kernel (v2).

Sharding: batch*head parallel across 8 cores. Core c handles batch b=c//4 and
global heads [4*(c%4), 4*(c%4)+4). Each core computes its 4 heads' dual-path
attention and the partial final projection (its 256 rows of out_w); the host
sums the 4 partials per batch (fp32) and adds out_b.

v2 layout/claims (all SBUF operands bf16, PSUM/normalization math fp32):
  - All weights prefetched to SBUF up front (two HWDGE queues), so phase A
    never stalls on DMA.
  - q^T/k^T stored as [128, 2, T]: head pair hp stacked (even head rows 0:64,
    odd 64:128, no zero padding).
  - Phase B per head-pair: std logits via two concurrent K=64 row-tiled MMs
    (tile_position (0,0)/(64,0)); geo logits via two concurrent K=6 MMs at
    32-strips. Diagonal k-tiles narrowed to the live columns ([128m:512]),
    which also shrinks exp and reduces causal masking to one 128-wide
    affine_select over both heads.
  - exp on ACT per (kt, path): [128, 2head, w] PSUM->SBUF bf16; U = [v|1]^T P
    accumulated in PSUM (denominator in row 64 for free).
  - Combine: denominator rows copied p64->p64 (DVE), DMA-relocated to
    partitions 0:4, one reciprocal + gate multiply on [4,512], DMA
    row-broadcast to [64,2,512], two TT muls + adds; odd head's combined
    half DMA-shifted to partitions 64:128 so the final projection runs as
    K=128 head-pair packed MMs.
  - Final projection: 2 MMs per (qt,et) tile, bf16 partial out; host sums.
"""

import os
import numpy as np
import ml_dtypes

import concourse.bass as bass
from concourse import bacc
import concourse.mybir as mybir
import concourse.tile as tile
from concourse.bass_utils import run_bass_kernel_spmd

D, H, B, T = 1024, 16, 2, 2048
DH = 64          # head dim
NH = 4           # heads per core
NHP = 2          # head pairs per core
NCORES = 8
QB = 512         # q block width
KT = 128         # k tile height
NQB = T // QB    # 4
F32 = mybir.dt.float32
BF = mybir.dt.bfloat16

PAIRS4 = [(0, 1), (0, 2), (0, 3), (1, 2), (1, 3), (2, 3)]
SIGMA = [1.0, -1.0, 1.0, 1.0, -1.0, 1.0]

TRACE = False            # set by test harness for profiling runs
LAST_RESULT = None       # BassKernelResults of last run (for exec_time_ns)


def _build_nc():
    nc = bacc.Bacc("TRN2", target_bir_lowering=False, debug=False)

    # ---- DRAM I/O ----
    d_xT = nc.dram_tensor("xT", [D, T], BF, kind="ExternalInput")
    d_wq = nc.dram_tensor("wq", [D, 256], BF, kind="ExternalInput")
    d_wk = nc.dram_tensor("wk", [D, 256], BF, kind="ExternalInput")
    d_wv = nc.dram_tensor("wv", [D, 256], BF, kind="ExternalInput")
    d_wgv = nc.dram_tensor("wgv", [D, 256], BF, kind="ExternalInput")
    d_wla = nc.dram_tensor("wla", [D, 64], BF, kind="ExternalInput")
    d_wlb = nc.dram_tensor("wlb", [D, 64], BF, kind="ExternalInput")
    d_wlc = nc.dram_tensor("wlc", [D, 64], BF, kind="ExternalInput")
    d_wld = nc.dram_tensor("wld", [D, 64], BF, kind="ExternalInput")
    d_wgate = nc.dram_tensor("wgate", [D, 16], BF, kind="ExternalInput")
    d_outw2 = nc.dram_tensor("outw2", [128, 2 * D], BF, kind="ExternalInput")
    d_bq = nc.dram_tensor("bq", [256, 1], F32, kind="ExternalInput")
    d_bk = nc.dram_tensor("bk", [256, 1], F32, kind="ExternalInput")
    d_bv = nc.dram_tensor("bv", [1, 256], BF, kind="ExternalInput")
    d_bgv = nc.dram_tensor("bgv", [1, 256], BF, kind="ExternalInput")
    d_bgate = nc.dram_tensor("bgate", [16, 1], F32, kind="ExternalInput")
    d_sbc = nc.dram_tensor("sbc", [64, 1], F32, kind="ExternalInput")
    d_ssel = nc.dram_tensor("ssel", [128, 64], BF, kind="ExternalInput")
    d_gsel = nc.dram_tensor("gsel", [16, 1], mybir.dt.float32r, kind="ExternalInput")
    d_vones = nc.dram_tensor("vones", [128, 64], BF, kind="ExternalInput")
    d_partial = nc.dram_tensor("partial", [T, D], BF, kind="ExternalOutput")
    d_acr = nc.dram_tensor("acr", [32, QB], BF, kind="Internal")

    AF = mybir.ActivationFunctionType
    OP = mybir.AluOpType

    with tile.TileContext(nc, linearize=bool(int(os.environ.get('KLIN', '0')))) as tc:
        with (
            tc.tile_pool(name="wC", bufs=1) as wC,
            tc.tile_pool(name="pers", bufs=1) as pers,
            tc.tile_pool(name="wA", bufs=1) as wA,
        ):
            # ---------- weight prefetch ----------
            # lines weights first (phase A1 starts on them), then the fat
            # projection weights; xT itself streams on the gpsimd queue.
            wl_sb = {}
            for i, dw in enumerate((d_wla, d_wlb, d_wlc, d_wld)):
                t = wA.tile([128, 8, 64], BF, name=f"wl{i}")
                eng = nc.sync if i % 2 == 0 else nc.scalar
                eng.dma_start(t[:], dw[:].rearrange("(k p) c -> p k c", p=128))
                wl_sb[i] = t
            wq_sb = wA.tile([128, 8, 256], BF)
            wk_sb = wA.tile([128, 8, 256], BF)
            wvg_sb = wA.tile([128, 8, 512], BF)
            nc.sync.dma_start(wq_sb[:], d_wq[:].rearrange("(k p) c -> p k c", p=128))
            nc.scalar.dma_start(wk_sb[:], d_wk[:].rearrange("(k p) c -> p k c", p=128))
            nc.sync.dma_start(wvg_sb[:, :, 0:256],
                              d_wv[:].rearrange("(k p) c -> p k c", p=128))
            nc.scalar.dma_start(wvg_sb[:, :, 256:512],
                                d_wgv[:].rearrange("(k p) c -> p k c", p=128))
            outw2_sb = wC.tile([128, 2, D], BF)
            nc.scalar.dma_start(
                outw2_sb[:], d_outw2[:].rearrange("p (h e) -> p h e", h=2))
            wgate_sb = wA.tile([128, 8, 16], BF)
            nc.sync.dma_start(wgate_sb[:], d_wgate[:].rearrange("(k p) c -> p k c", p=128))
            bq_sb = wA.tile([128, 2], F32)
            bk_sb = wA.tile([128, 2], F32)
            nc.sync.dma_start(bq_sb[:], d_bq[:].rearrange("(m p) o -> p (m o)", p=128))
            nc.scalar.dma_start(bk_sb[:], d_bk[:].rearrange("(m p) o -> p (m o)", p=128))
            # bias rows broadcast across partitions for the v/gv TT-add
            bvb = wA.tile([128, 256], BF)
            bgvb = wA.tile([128, 256], BF)
            nc.sync.dma_start(bvb[:], bass.AP(tensor=d_bv, offset=0,
                                              ap=[[0, 128], [1, 256]]))
            nc.scalar.dma_start(bgvb[:], bass.AP(tensor=d_bgv, offset=0,
                                                 ap=[[0, 128], [1, 256]]))
            bgate_sb = wA.tile([16, 1], F32)
            nc.sync.dma_start(bgate_sb[:], d_bgate[:])
            sbc_sb = wA.tile([64, 1], F32)
            nc.scalar.dma_start(sbc_sb[:], d_sbc[:])
            ssel_sb = wA.tile([128, 64], BF)
            nc.sync.dma_start(ssel_sb[:], d_ssel[:])
            gsel_sb = wA.tile([16, 1], mybir.dt.float32r)
            nc.scalar.dma_start(gsel_sb[:], d_gsel[:])

            # ---------- persistent B-phase tensors ----------
            qT = pers.tile([128, 2, T], BF)      # pair hp: even 0:64, odd 64:128
            kTp2 = pers.tile([128, 2, T], BF)
            vplus = pers.tile([128, 16, NH * 65], BF)
            gvplus = pers.tile([128, 16, NH * 65], BF)
            jwT = pers.tile([128, T], BF)        # head h lines at [32h, 32h+6)
            rlT = pers.tile([128, T], BF)
            comb = pers.tile([128, 2, T], BF)    # pair hp, even 0:64 odd 64:128
            G4 = pers.tile([4, T], F32)          # rows: 1-g, 1-g, g, g
            g_row = pers.tile([1, T], F32)
            g1m_row = pers.tile([1, T], F32)

            # ones columns of v/gv (col 64 of each head's 65-wide group)
            nc.sync.dma_start(
                vplus[:].rearrange("p t (h c) -> p t h c", c=65)[:, :, :, 64:65],
                d_vones[:].rearrange("p (t h) -> p t h", h=NH))
            nc.scalar.dma_start(
                gvplus[:].rearrange("p t (h c) -> p t h c", c=65)[:, :, :, 64:65],
                d_vones[:].rearrange("p (t h) -> p t h", h=NH))

            with tc.tile_pool(name="xp", bufs=1) as xp:
                xT_sb = xp.tile([128, 8, T], BF)
                for ko in range(8):
                    nc.gpsimd.dma_start(
                        out=xT_sb[:, ko, :], in_=d_xT[128 * ko:128 * (ko + 1), :])

                with tc.tile_pool(name="psA", bufs=6,
                                  space=bass.MemorySpace.PSUM) as psA:
                    # ---------- A1: Pluecker lines ----------
                    with tc.tile_pool(name="lines", bufs=1) as lnp:
                        def _project(dst, wsb):
                            pss = [psA.tile([64, QB], F32, tag="s",
                                            name=f"lps{tb}")
                                   for tb in range(NQB)]
                            for kc in range(8):
                                for tb in range(NQB):
                                    nc.tensor.matmul(
                                        pss[tb][:], wsb[:, kc, :],
                                        xT_sb[:, kc, QB * tb:QB * (tb + 1)],
                                        start=(kc == 0), stop=(kc == 7))
                            for tb in range(NQB):
                                nc.scalar.copy(dst[:, QB * tb:QB * (tb + 1)],
                                               pss[tb][:])

                        def _product(t, wa, wb):
                            PX = lnp.tile([64, T], F32, tag="a", name="PX")
                            PY = lnp.tile([64, T], F32, tag="b", name="PY")
                            _project(PX, wa)
                            _project(PY, wb)
                            # rows 0:32 write-path (uses x_prev on the A side)
                            nc.vector.tensor_mul(
                                t[0:32, 1:T], PX[0:32, 0:T - 1], PY[0:32, 1:T])
                            nc.gpsimd.affine_select(
                                out=t[0:32, 0:1], in_=t[0:32, 1:2],
                                compare_op=OP.is_gt, fill=0.0,
                                base=0, pattern=[[0, 1]], channel_multiplier=0)
                            nc.vector.tensor_mul(
                                t[32:64, :], PX[32:64, :], PY[32:64, :])

                        t1 = lnp.tile([64, T], F32, tag="e")
                        t2 = lnp.tile([64, T], F32, tag="f")
                        t2b = lnp.tile([128, T], BF, tag="g")   # squares, padded
                        nc.vector.memset(t2b[64:128, :], 0.0)
                        _product(t1, wl_sb[0], wl_sb[1])
                        _product(t2, wl_sb[2], wl_sb[3])
                        nc.vector.tensor_sub(t1[:], t1[:], t2[:])   # lines_u

                        # q/k projections here: their MMs fill the PE while
                        # the DVE/ACT line-normalization tail runs
                        for mc in range(2):
                            for (wsb, bias, dst) in ((wq_sb, bq_sb, qT),
                                                     (wk_sb, bk_sb, kTp2)):
                                for tb in range(NQB):
                                    ps = psA.tile([128, QB], F32, tag="s")
                                    for kc in range(8):
                                        nc.tensor.matmul(
                                            ps[:],
                                            wsb[:, kc, 128 * mc:128 * (mc + 1)],
                                            xT_sb[:, kc, QB * tb:QB * (tb + 1)],
                                            start=(kc == 0), stop=(kc == 7))
                                    nc.vector.tensor_scalar_add(
                                        out=dst[:, mc, QB * tb:QB * (tb + 1)],
                                        in0=ps[:], scalar1=bias[:, mc:mc + 1])

                        nc.scalar.square(t2b[0:64, :], t1[:])       # squares bf16

                        ssq = lnp.tile([64, T], F32, tag="a")
                        for tb in range(NQB):
                            ps = psA.tile([64, QB], F32, tag="s")
                            nc.tensor.matmul(
                                ps[:], ssel_sb[:],
                                t2b[:, QB * tb:QB * (tb + 1)],
                                start=True, stop=True)
                            nc.vector.tensor_scalar_max(
                                out=ssq[:, QB * tb:QB * (tb + 1)], in0=ps[:],
                                scalar1=1e-24)
                        rt = lnp.tile([64, T], F32, tag="b")
                        nc.scalar.sqrt(rt[:], ssq[:])
                        inv = lnp.tile([64, T], F32, tag="a")
                        nc.vector.reciprocal_approx_fast(out=inv[:], in_=rt[:])
                        # fold inc_scale into read-line norms (rows 0:32 are 1.0)
                        nc.vector.tensor_scalar_mul(
                            out=inv[:], in0=inv[:], scalar1=sbc_sb[:, 0:1])
                        lbf = lnp.tile([64, T], BF, tag="f")
                        nc.vector.tensor_mul(lbf[:], t1[:], inv[:])  # lines_n

                        # scatter to 32-aligned per-head layout via DMA
                        for h in range(NH):
                            eng = nc.sync if h % 2 == 0 else nc.scalar
                            eng.dma_start(
                                out=jwT[32 * h:32 * h + 6, :],
                                in_=lbf[6 * h:6 * h + 6, :])
                            eng.dma_start(
                                out=rlT[32 * h:32 * h + 6, :],
                                in_=lbf[32 + 6 * h:32 + 6 * h + 6, :])


                    # ---------- A2b: v / geo_v (one N=512 pass) ----------
                    for ti in range(16):
                        ps = psA.tile([128, QB], F32, tag="s")
                        for kc in range(8):
                            nc.tensor.matmul(
                                ps[:],
                                xT_sb[:, kc, 128 * ti:128 * (ti + 1)],
                                wvg_sb[:, kc, :],
                                start=(kc == 0), stop=(kc == 7))
                        for ci, (dst, biasb) in enumerate(((vplus, bvb),
                                                           (gvplus, bgvb))):
                            nc.vector.tensor_add(
                                dst[:, ti, :].rearrange(
                                    "p (h c) -> p h c", c=65)[:, :, 0:64],
                                ps[:, 256 * ci:256 * (ci + 1)].rearrange(
                                    "p (h c) -> p h c", c=64),
                                biasb[:].rearrange("p (h c) -> p h c", c=64))

                    # ---------- A2c: gate ----------
                    for tb in range(NQB):
                        ps = psA.tile([16, QB], F32, tag="s")
                        for kc in range(8):
                            nc.tensor.matmul(
                                ps[:], wgate_sb[:, kc, :],
                                xT_sb[:, kc, QB * tb:QB * (tb + 1)],
                                start=(kc == 0), stop=(kc == 7))
                        gsig = wA.tile([16, QB], mybir.dt.float32r, tag="gs",
                                       name=f"gsig{tb}")
                        nc.scalar.activation(
                            out=gsig[:], in_=ps[:], func=AF.Sigmoid,
                            bias=bgate_sb[:, 0:1], scale=1.0)
                        psg = psA.tile([1, QB], F32, tag="s")
                        nc.tensor.matmul(psg[:], gsel_sb[:], gsig[:],
                                         start=True, stop=True)
                        nc.vector.tensor_copy(
                            g_row[:, QB * tb:QB * (tb + 1)], psg[:])
                    nc.vector.tensor_scalar(
                        out=g1m_row[:], in0=g_row[:],
                        scalar1=-1.0, scalar2=1.0, op0=OP.mult, op1=OP.add)
                    # assemble G4 rows (1-g, 1-g, g, g) via relocate DMAs
                    nc.sync.dma_start(G4[0:1, :], g1m_row[:])
                    nc.scalar.dma_start(G4[1:2, :], g1m_row[:])
                    nc.sync.dma_start(G4[2:3, :], g_row[:])
                    nc.scalar.dma_start(G4[3:4, :], g_row[:])

            # ---------- Phase B: dual-path attention ----------
            with (
                tc.tile_pool(name="psU", bufs=2, space=bass.MemorySpace.PSUM) as psU,
                tc.tile_pool(name="psL", bufs=2, space=bass.MemorySpace.PSUM) as psL,
                tc.tile_pool(name="pp", bufs=8) as pp,
                tc.tile_pool(name="cbp", bufs=4) as cbp,
                tc.tile_pool(name="rowp", bufs=4) as rowp,
            ):
                for j in range(NQB):
                    for hp in range(NHP):
                        he, ho = 2 * hp, 2 * hp + 1   # local head indices
                        nkt = 4 * (j + 1)
                        Us = psU.tile([65, 2, QB], F32, tag="u", name="Us")
                        Ug = psU.tile([65, 2, QB], F32, tag="u", name="Ug")
                        for kt in range(nkt):
                            m = kt - 4 * j
                            off = KT * m if m >= 0 else 0
                            lsl = slice(off, QB)
                            qsl = slice(QB * j + off, QB * (j + 1))
                            ksl = slice(KT * kt, KT * (kt + 1))
                            Ls = psL.tile([128, 2, QB], F32, tag="L", name="Ls")
                            Lg = psL.tile([128, 2, QB], F32, tag="L", name="Lg")
                            nc.tensor.matmul(
                                Ls[:, 0, lsl], kTp2[0:64, hp, ksl],
                                qT[0:64, hp, qsl],
                                start=True, stop=True, tile_position=(0, 0))
                            nc.tensor.matmul(
                                Ls[:, 1, lsl], kTp2[64:128, hp, ksl],
                                qT[64:128, hp, qsl],
                                start=True, stop=True, tile_position=(64, 0))
                            nc.tensor.matmul(
                                Lg[:, 0, lsl], jwT[32 * he:32 * he + 6, ksl],
                                rlT[32 * he:32 * he + 6, qsl],
                                start=True, stop=True,
                                tile_position=(32 * he, 0))
                            nc.tensor.matmul(
                                Lg[:, 1, lsl], jwT[32 * ho:32 * ho + 6, ksl],
                                rlT[32 * ho:32 * ho + 6, qsl],
                                start=True, stop=True,
                                tile_position=(32 * ho, 0))
                            Ps = pp.tile([128, 2, QB], BF, tag="P", name="Ps")
                            Pg = pp.tile([128, 2, QB], BF, tag="P", name="Pg")
                            nc.scalar.activation(Ps[:, :, lsl], Ls[:, :, lsl],
                                                 AF.Exp)
                            nc.scalar.activation(Pg[:, :, lsl], Lg[:, :, lsl],
                                                 AF.Exp)
                            if m >= 0:
                                # boundary 128 cols: keep where qc - kr >= 0
                                for Px in (Ps, Pg):
                                    nc.gpsimd.affine_select(
                                        out=Px[:, :, off:off + KT],
                                        in_=Px[:, :, off:off + KT],
                                        compare_op=OP.is_ge, fill=0.0,
                                        base=0, pattern=[[0, 2], [1, KT]],
                                        channel_multiplier=-1)
                            for (U, Pt, vt) in ((Us, Ps, vplus), (Ug, Pg, gvplus)):
                                nc.tensor.matmul(
                                    U[:, 0, lsl], vt[:, kt, 65 * he:65 * he + 65],
                                    Pt[:, 0, lsl],
                                    start=(kt == 0), stop=(kt == nkt - 1))
                                nc.tensor.matmul(
                                    U[:, 1, lsl], vt[:, kt, 65 * ho:65 * ho + 65],
                                    Pt[:, 1, lsl],
                                    start=(kt == 0), stop=(kt == nkt - 1))

                        # ---- combine ----
                        # copy U out of PSUM first so the next block's U MMs
                        # aren't blocked on the (long) normalization chain;
                        # row 64 of the copies is the denominator for free.
                        qsl = slice(QB * j, QB * (j + 1))
                        Usb = cbp.tile([65, 2, QB], F32, tag="ub", name="Usb")
                        Ugb = cbp.tile([65, 2, QB], F32, tag="ub", name="Ugb")
                        nc.vector.tensor_copy(Usb[:], Us[:, :, :])
                        nc.vector.tensor_copy(Ugb[:], Ug[:, :, :])
                        Dc = rowp.tile([4, QB], F32, tag="d")
                        nc.sync.dma_start(Dc[0:2, :], Usb[64:65, :, :])
                        nc.scalar.dma_start(Dc[2:4, :], Ugb[64:65, :, :])
                        rDc = rowp.tile([4, QB], F32, tag="d")
                        nc.vector.reciprocal_approx_fast(out=rDc[:], in_=Dc[:])
                        ac_ = rowp.tile([4, QB], BF, tag="d")
                        nc.vector.tensor_mul(ac_[:], rDc[:], G4[:, qsl])
                        aB = cbp.tile([64, 2, QB], BF, tag="ab", name="aB")
                        bB = cbp.tile([64, 2, QB], BF, tag="ab", name="bB")
                        # SBUF rows can't partition-broadcast directly;
                        # bounce through DRAM scratch (rows r*8 + hp*4 + j)
                        idx = hp * 4 + j
                        nc.sync.dma_start(
                            d_acr[:].rearrange(
                                "(r i) c -> r i c", i=8)[:, idx, :],
                            ac_[:])
                        for r, (dst, sl) in enumerate((
                                (aB, 0), (aB, 1), (bB, 0), (bB, 1))):
                            eng = nc.sync if r % 2 == 0 else nc.scalar
                            eng.dma_start(
                                dst[:, sl, :],
                                bass.AP(tensor=d_acr, offset=(r * 8 + idx) * QB,
                                        ap=[[0, 64], [1, QB]]))
                        u1 = cbp.tile([64, 2, QB], BF, tag="ab", name="u1")
                        u2 = cbp.tile([64, 2, QB], BF, tag="ab", name="u2")
                        nc.vector.tensor_mul(u1[:], Usb[0:64, :, :], aB[:])
                        nc.vector.tensor_mul(u2[:], Ugb[0:64, :, :], bB[:])
                        nc.vector.tensor_add(
                            comb[0:64, hp, qsl], u1[:, 0, :], u2[:, 0, :])
                        codd = cbp.tile([64, QB], BF, tag="co")
                        nc.vector.tensor_add(codd[:], u1[:, 1, :], u2[:, 1, :])
                        nc.sync.dma_start(comb[64:128, hp, qsl], codd[:])

            # ---------- Phase C: final projection ----------
            with (
                tc.tile_pool(name="psC", bufs=4, space=bass.MemorySpace.PSUM) as psC,
                tc.tile_pool(name="outs", bufs=4) as op_,
            ):
                for qt in range(16):
                    for et in range(2):
                        ps = psC.tile([128, QB], F32, tag="s")
                        for hpc in range(2):
                            nc.tensor.matmul(
                                ps[:],
                                comb[:, hpc, 128 * qt:128 * (qt + 1)],
                                outw2_sb[:, hpc, QB * et:QB * (et + 1)],
                                start=(hpc == 0), stop=(hpc == 1))
                        ot = op_.tile([128, QB], BF, tag="o")
                        if et == 0:
                            nc.scalar.copy(ot[:], ps[:])
                        else:
                            nc.vector.tensor_copy(ot[:], ps[:])
                        eng = nc.sync if et == 0 else nc.scalar
                        eng.dma_start(
                            d_partial[128 * qt:128 * (qt + 1),
                                      QB * et:QB * (et + 1)],
                            ot[:])


    nc.compile()
    return nc


_nc_cache = None


def _get_nc():
    global _nc_cache
    if _nc_cache is None:
        _nc_cache = _build_nc()
    return _nc_cache


def _prep_core_inputs(inputs, core):
    b = core // 4
    h0 = (core % 4) * 4
    f = np.float32
    bf = ml_dtypes.bfloat16
    qkv_w, qkv_b = inputs['qkv_w'], inputs['qkv_b']
    scale = DH ** -0.5
    s = slice(h0 * DH, h0 * DH + NH * DH)
    ac = np.ascontiguousarray

    # Operand layout: 64 cols = [write(24)+pad8 | read(24)+pad8]; A/C from w1
    # (shifted = x_prev side), B/D from the w2/read counterparts.
    WLA = np.zeros((D, 64), f); WLB = np.zeros((D, 64), f)
    WLC = np.zeros((D, 64), f); WLD = np.zeros((D, 64), f)
    w1w, w2w = inputs['w1_write'], inputs['w2_write']
    w1r, w2r = inputs['w1_read'], inputs['w2_read']
    for h in range(NH):
        gh = h0 + h
        for jj in range(6):
            i_, j_ = PAIRS4[5 - jj]
            WLA[:, 0 + h * 6 + jj] = w1w[:, gh * 4 + i_] * SIGMA[jj]    # A_w
            WLB[:, 0 + h * 6 + jj] = w2w[:, gh * 4 + j_]                # B_w
            WLC[:, 0 + h * 6 + jj] = w1w[:, gh * 4 + j_] * SIGMA[jj]    # C_w
            WLD[:, 0 + h * 6 + jj] = w2w[:, gh * 4 + i_]                # D_w
        for pp in range(6):
            i_, j_ = PAIRS4[pp]
            WLA[:, 32 + h * 6 + pp] = w1r[:, gh * 4 + i_]               # A_r
            WLB[:, 32 + h * 6 + pp] = w2r[:, gh * 4 + j_]               # B_r
            WLC[:, 32 + h * 6 + pp] = w1r[:, gh * 4 + j_]               # C_r
            WLD[:, 32 + h * 6 + pp] = w2r[:, gh * 4 + i_]               # D_r

    ssel = np.zeros((128, 64), f)
    for half in (0, 32):
        for h in range(NH):
            g = slice(half + 6 * h, half + 6 * h + 6)
            ssel[g, g] = 1.0
    sbc = np.ones((64, 1), f)
    sbc[32:56, 0] = np.repeat(inputs['inc_scale'][h0:h0 + NH], 6).astype(f)

    # out_w rows packed as head pairs: rows 0:64 = even head, 64:128 = odd
    ow = np.asarray(inputs['out_w'], f)
    outw2 = np.zeros((128, 2, D), f)
    for hp in range(2):
        outw2[0:64, hp, :] = ow[(h0 + 2 * hp) * DH:(h0 + 2 * hp + 1) * DH, :]
        outw2[64:128, hp, :] = ow[(h0 + 2 * hp + 1) * DH:(h0 + 2 * hp + 2) * DH, :]

    return {
        'xT': ac(np.asarray(inputs['x'][b], f).T).astype(bf),
        'wq': ac((qkv_w[:, 0 * D:1 * D][:, s] * scale).astype(f)).astype(bf),
        'wk': ac(qkv_w[:, 1 * D:2 * D][:, s].astype(f)).astype(bf),
        'wv': ac(qkv_w[:, 2 * D:3 * D][:, s].astype(f)).astype(bf),
        'wgv': ac(inputs['geo_w'][:, s].astype(f)).astype(bf),
        'wla': WLA.astype(bf), 'wlb': WLB.astype(bf),
        'wlc': WLC.astype(bf), 'wld': WLD.astype(bf),
        'wgate': ac(inputs['gate_w'].astype(f)).astype(bf),
        'outw2': ac(outw2.reshape(128, 2 * D)).astype(bf),
        'bq': ac((qkv_b[0 * D:1 * D][s] * scale).astype(f).reshape(256, 1)),
        'bk': ac(qkv_b[1 * D:2 * D][s].astype(f).reshape(256, 1)),
        'bv': ac(qkv_b[2 * D:3 * D][s].astype(f).reshape(1, 256)).astype(bf),
        'bgv': ac(inputs['geo_b'][s].astype(f).reshape(1, 256)).astype(bf),
        'bgate': ac(inputs['gate_b'].astype(f).reshape(16, 1)),
        'sbc': sbc,
        'ssel': ssel.astype(bf),
        'gsel': np.full((16, 1), 1.0 / 16.0, f),
        'vones': np.ones((128, 64), f).astype(bf),
    }


def kernel(**inputs):
    global LAST_RESULT
    inputs = {k: np.asarray(v) for k, v in inputs.items()}
    nc = _get_nc()
    in_maps = [_prep_core_inputs(inputs, c) for c in range(NCORES)]
    res = run_bass_kernel_spmd(nc, in_maps, core_ids=list(range(NCORES)),
                               trace=TRACE)
    LAST_RESULT = res
    out = np.zeros((B, T, D), np.float32)
    for c in range(NCORES):
        out[c // 4] += np.asarray(res.results[c]['partial'], np.float32)
    out += np.asarray(inputs['out_b'], np.float32)[None, None, :]
    return out


# revision 29
# speedup vs baseline: 1.0368x; 1.0368x over previous
"""DualPathAttention Trainium2 Bass kernel (v2).

Sharding: batch*head parallel across 8 cores. Core c handles batch b=c//4 and
global heads [4*(c%4), 4*(c%4)+4). Each core computes its 4 heads' dual-path
attention and the partial final projection (its 256 rows of out_w); the host
sums the 4 partials per batch (fp32) and adds out_b.

v2 layout/claims (all SBUF operands bf16, PSUM/normalization math fp32):
  - All weights prefetched to SBUF up front (two HWDGE queues), so phase A
    never stalls on DMA.
  - q^T/k^T stored as [128, 2, T]: head pair hp stacked (even head rows 0:64,
    odd 64:128, no zero padding).
  - Phase B per head-pair: std logits via two concurrent K=64 row-tiled MMs
    (tile_position (0,0)/(64,0)); geo logits via two concurrent K=6 MMs at
    32-strips. Diagonal k-tiles narrowed to the live columns ([128m:512]),
    which also shrinks exp and reduces causal masking to one 128-wide
    affine_select over both heads.
  - exp on ACT per (kt, path): [128, 2head, w] PSUM->SBUF bf16; U = [v|1]^T P
    accumulated in PSUM (denominator in row 64 for free).
  - Combine: denominator rows copied p64->p64 (DVE), DMA-relocated to
    partitions 0:4, one reciprocal + gate multiply on [4,512], DMA
    row-broadcast to [64,2,512], two TT muls + adds; odd head's combined
    half DMA-shifted to partitions 64:128 so the final projection runs as
    K=128 head-pair packed MMs.
  - Final projection: 2 MMs per (qt,et) tile, bf16 partial out; host sums.
"""

import os
import numpy as np
import ml_dtypes

import concourse.bass as bass
from concourse import bacc
import concourse.mybir as mybir
import concourse.tile as tile
from concourse.bass_utils import run_bass_kernel_spmd

D, H, B, T = 1024, 16, 2, 2048
DH = 64          # head dim
NH = 4           # heads per core
NHP = 2          # head pairs per core
NCORES = 8
QB = 512         # q block width
KT = 128         # k tile height
NQB = T // QB    # 4
F32 = mybir.dt.float32
BF = mybir.dt.bfloat16

PAIRS4 = [(0, 1), (0, 2), (0, 3), (1, 2), (1, 3), (2, 3)]
SIGMA = [1.0, -1.0, 1.0, 1.0, -1.0, 1.0]

TRACE = False            # set by test harness for profiling runs
LAST_RESULT = None       # BassKernelResults of last run (for exec_time_ns)


def _build_nc():
    nc = bacc.Bacc("TRN2", target_bir_lowering=False, debug=False)

    # ---- DRAM I/O ----
    d_xT = nc.dram_tensor("xT", [D, T], BF, kind="ExternalInput")
    d_wq = nc.dram_tensor("wq", [D, 256], BF, kind="ExternalInput")
    d_wk = nc.dram_tensor("wk", [D, 256], BF, kind="ExternalInput")
    d_wv = nc.dram_tensor("wv", [D, 256], BF, kind="ExternalInput")
    d_wgv = nc.dram_tensor("wgv", [D, 256], BF, kind="ExternalInput")
    d_wla = nc.dram_tensor("wla", [D, 80], BF, kind="ExternalInput")
    d_wlb = nc.dram_tensor("wlb", [D, 64], BF, kind="ExternalInput")
    d_wlc = nc.dram_tensor("wlc", [D, 64], BF, kind="ExternalInput")
    d_wld = nc.dram_tensor("wld", [D, 64], BF, kind="ExternalInput")
    d_outw2 = nc.dram_tensor("outw2", [128, 2 * D], BF, kind="ExternalInput")
    d_bq = nc.dram_tensor("bq", [256, 1], F32, kind="ExternalInput")
    d_bk = nc.dram_tensor("bk", [256, 1], F32, kind="ExternalInput")
    d_bv = nc.dram_tensor("bv", [1, 256], BF, kind="ExternalInput")
    d_bgv = nc.dram_tensor("bgv", [1, 256], BF, kind="ExternalInput")
    d_bgate = nc.dram_tensor("bgate", [16, 1], F32, kind="ExternalInput")
    d_sbc = nc.dram_tensor("sbc", [64, 1], F32, kind="ExternalInput")
    d_ssel = nc.dram_tensor("ssel", [128, 64], BF, kind="ExternalInput")
    d_gsel = nc.dram_tensor("gsel", [16, 1], mybir.dt.float32r, kind="ExternalInput")
    d_vones = nc.dram_tensor("vones", [128, 64], BF, kind="ExternalInput")
    d_partial = nc.dram_tensor("partial", [T, D], BF, kind="ExternalOutput")
    d_acr = nc.dram_tensor("acr", [32, QB], BF, kind="Internal")

    AF = mybir.ActivationFunctionType
    OP = mybir.AluOpType

    with tile.TileContext(nc, linearize=bool(int(os.environ.get('KLIN', '0')))) as tc:
        with (
            tc.tile_pool(name="wC", bufs=1) as wC,
            tc.tile_pool(name="pers", bufs=1) as pers,
            tc.tile_pool(name="wA", bufs=1) as wA,
        ):
            # ---------- weight prefetch ----------
            # lines weights first (phase A1 starts on them), then the fat
            # projection weights; xT itself streams on the gpsimd queue.
            wl_sb = {}
            for i, dw in enumerate((d_wla, d_wlb, d_wlc, d_wld)):
                nc_ = 80 if i == 0 else 64
                t = wA.tile([128, 8, nc_], BF, name=f"wl{i}")
                eng = nc.sync if i % 2 == 0 else nc.scalar
                eng.dma_start(t[:], dw[:].rearrange("(k p) c -> p k c", p=128))
                wl_sb[i] = t
            wq_sb = wA.tile([128, 8, 256], BF)
            wk_sb = wA.tile([128, 8, 256], BF)
            wvg_sb = wA.tile([128, 8, 512], BF)
            nc.sync.dma_start(wq_sb[:], d_wq[:].rearrange("(k p) c -> p k c", p=128))
            nc.scalar.dma_start(wk_sb[:], d_wk[:].rearrange("(k p) c -> p k c", p=128))
            nc.sync.dma_start(wvg_sb[:, :, 0:256],
                              d_wv[:].rearrange("(k p) c -> p k c", p=128))
            nc.scalar.dma_start(wvg_sb[:, :, 256:512],
                                d_wgv[:].rearrange("(k p) c -> p k c", p=128))
            outw2_sb = wC.tile([128, 2, D], BF)
            nc.scalar.dma_start(
                outw2_sb[:], d_outw2[:].rearrange("p (h e) -> p h e", h=2))
            bq_sb = wA.tile([128, 2], F32)
            bk_sb = wA.tile([128, 2], F32)
            nc.sync.dma_start(bq_sb[:], d_bq[:].rearrange("(m p) o -> p (m o)", p=128))
            nc.scalar.dma_start(bk_sb[:], d_bk[:].rearrange("(m p) o -> p (m o)", p=128))
            # bias rows broadcast across partitions for the v/gv TT-add
            bvb = wA.tile([128, 256], BF)
            bgvb = wA.tile([128, 256], BF)
            nc.sync.dma_start(bvb[:], bass.AP(tensor=d_bv, offset=0,
                                              ap=[[0, 128], [1, 256]]))
            nc.scalar.dma_start(bgvb[:], bass.AP(tensor=d_bgv, offset=0,
                                                 ap=[[0, 128], [1, 256]]))
            bgate_sb = wA.tile([80, 1], F32)
            nc.sync.dma_start(bgate_sb[64:80, :], d_bgate[:])
            sbc_sb = wA.tile([64, 1], F32)
            nc.scalar.dma_start(sbc_sb[:], d_sbc[:])
            ssel_sb = wA.tile([128, 64], BF)
            nc.sync.dma_start(ssel_sb[:], d_ssel[:])
            gsel_sb = wA.tile([80, 1], mybir.dt.float32r)
            nc.scalar.dma_start(gsel_sb[64:80, :], d_gsel[:])

            # ---------- persistent B-phase tensors ----------
            qT = pers.tile([128, 2, T], BF)      # pair hp: even 0:64, odd 64:128
            kTp2 = pers.tile([128, 2, T], BF)
            vplus = pers.tile([128, 16, NH * 65], BF)
            gvplus = pers.tile([128, 16, NH * 65], BF)
            jwT = pers.tile([128, T], BF)        # head h lines at [32h, 32h+6)
            rlT = pers.tile([128, T], BF)
            comb = pers.tile([128, 2, T], BF)    # pair hp, even 0:64 odd 64:128
            G4 = pers.tile([4, T], F32)          # rows: 1-g, 1-g, g, g
            g_row = pers.tile([1, T], F32)
            g1m_row = pers.tile([1, T], F32)

            # ones columns of v/gv (col 64 of each head's 65-wide group)
            nc.sync.dma_start(
                vplus[:].rearrange("p t (h c) -> p t h c", c=65)[:, :, :, 64:65],
                d_vones[:].rearrange("p (t h) -> p t h", h=NH))
            nc.scalar.dma_start(
                gvplus[:].rearrange("p t (h c) -> p t h c", c=65)[:, :, :, 64:65],
                d_vones[:].rearrange("p (t h) -> p t h", h=NH))

            with tc.tile_pool(name="xp", bufs=1) as xp:
                xT_sb = xp.tile([128, 8, T], BF)
                for ko in range(8):
                    nc.gpsimd.dma_start(
                        out=xT_sb[:, ko, :], in_=d_xT[128 * ko:128 * (ko + 1), :])

                with tc.tile_pool(name="psA", bufs=6,
                                  space=bass.MemorySpace.PSUM) as psA:
                    # ---------- A1: Pluecker lines ----------
                    with tc.tile_pool(name="lines", bufs=1) as lnp:
                        def _project(dst, wsb, mrows=64):
                            pss = [psA.tile([mrows, QB], F32, tag="s",
                                            name=f"lps{tb}")
                                   for tb in range(NQB)]
                            for kc in range(8):
                                for tb in range(NQB):
                                    nc.tensor.matmul(
                                        pss[tb][:], wsb[:, kc, :],
                                        xT_sb[:, kc, QB * tb:QB * (tb + 1)],
                                        start=(kc == 0), stop=(kc == 7))
                            for tb in range(NQB):
                                nc.scalar.copy(dst[:, QB * tb:QB * (tb + 1)],
                                               pss[tb][:])

                        def _product(t, wa, wb, mx=64):
                            PX = lnp.tile([mx, T], F32, tag="a", name="PX")
                            PY = lnp.tile([64, T], F32, tag="b", name="PY")
                            _project(PX, wa, mrows=mx)
                            _project(PY, wb)
                            # rows 0:32 write-path (uses x_prev on the A side)
                            nc.vector.tensor_mul(
                                t[0:32, 1:T], PX[0:32, 0:T - 1], PY[0:32, 1:T])
                            nc.gpsimd.affine_select(
                                out=t[0:32, 0:1], in_=t[0:32, 1:2],
                                compare_op=OP.is_gt, fill=0.0,
                                base=0, pattern=[[0, 1]], channel_multiplier=0)
                            nc.vector.tensor_mul(
                                t[32:64, :], PX[32:64, :], PY[32:64, :])
                            return PX

                        t1 = lnp.tile([64, T], F32, tag="e")
                        t2 = lnp.tile([64, T], F32, tag="f")
                        t2b = lnp.tile([128, T], BF, tag="g")   # squares, padded
                        nc.vector.memset(t2b[64:128, :], 0.0)
                        PXg = _product(t1, wl_sb[0], wl_sb[1], mx=80)
                        _product(t2, wl_sb[2], wl_sb[3])
                        # gate logits rode along in rows 64:80 of the wla
                        # projection; sigmoid + mean-MM from there
                        for tb in range(NQB):
                            gsig = wA.tile([80, QB], mybir.dt.float32r,
                                           tag="gs", name=f"gsig{tb}")
                            nc.scalar.activation(
                                out=gsig[64:80, :],
                                in_=PXg[64:80, QB * tb:QB * (tb + 1)],
                                func=AF.Sigmoid,
                                bias=bgate_sb[64:80, 0:1], scale=1.0)
                            psg = psA.tile([1, QB], F32, tag="s")
                            nc.tensor.matmul(psg[:], gsel_sb[64:80, :],
                                             gsig[64:80, :],
                                             start=True, stop=True,
                                             tile_position=(64, 0))
                            nc.vector.tensor_copy(
                                g_row[:, QB * tb:QB * (tb + 1)], psg[:])
                        nc.vector.tensor_scalar(
                            out=g1m_row[:], in0=g_row[:],
                            scalar1=-1.0, scalar2=1.0, op0=OP.mult, op1=OP.add)
                        nc.sync.dma_start(G4[0:1, :], g1m_row[:])
                        nc.scalar.dma_start(G4[1:2, :], g1m_row[:])
                        nc.sync.dma_start(G4[2:3, :], g_row[:])
                        nc.scalar.dma_start(G4[3:4, :], g_row[:])
                        nc.vector.tensor_sub(t1[:], t1[:], t2[:])   # lines_u

                        # q/k projections here: their MMs fill the PE while
                        # the DVE/ACT line-normalization tail runs
                        for mc in range(2):
                            for (wsb, bias, dst) in ((wq_sb, bq_sb, qT),
                                                     (wk_sb, bk_sb, kTp2)):
                                for tb in range(NQB):
                                    ps = psA.tile([128, QB], F32, tag="s")
                                    for kc in range(8):
                                        nc.tensor.matmul(
                                            ps[:],
                                            wsb[:, kc, 128 * mc:128 * (mc + 1)],
                                            xT_sb[:, kc, QB * tb:QB * (tb + 1)],
                                            start=(kc == 0), stop=(kc == 7))
                                    nc.vector.tensor_scalar_add(
                                        out=dst[:, mc, QB * tb:QB * (tb + 1)],
                                        in0=ps[:], scalar1=bias[:, mc:mc + 1])

                        nc.scalar.square(t2b[0:64, :], t1[:])       # squares bf16

                        ssq = lnp.tile([64, T], F32, tag="a")
                        for tb in range(NQB):
                            ps = psA.tile([64, QB], F32, tag="s")
                            nc.tensor.matmul(
                                ps[:], ssel_sb[:],
                                t2b[:, QB * tb:QB * (tb + 1)],
                                start=True, stop=True)
                            nc.vector.tensor_scalar_max(
                                out=ssq[:, QB * tb:QB * (tb + 1)], in0=ps[:],
                                scalar1=1e-24)
                        rt = lnp.tile([64, T], F32, tag="b")
                        nc.scalar.sqrt(rt[:], ssq[:])
                        inv = lnp.tile([64, T], F32, tag="a")
                        nc.vector.reciprocal_approx_fast(out=inv[:], in_=rt[:])
                        # fold inc_scale into read-line norms (rows 0:32 are 1.0)
                        nc.vector.tensor_scalar_mul(
                            out=inv[:], in0=inv[:], scalar1=sbc_sb[:, 0:1])
                        lbf = lnp.tile([64, T], BF, tag="f")
                        nc.vector.tensor_mul(lbf[:], t1[:], inv[:])  # lines_n

                        # scatter to 32-aligned per-head layout via DMA
                        for h in range(NH):
                            eng = nc.sync if h % 2 == 0 else nc.scalar
                            eng.dma_start(
                                out=jwT[32 * h:32 * h + 6, :],
                                in_=lbf[6 * h:6 * h + 6, :])
                            eng.dma_start(
                                out=rlT[32 * h:32 * h + 6, :],
                                in_=lbf[32 + 6 * h:32 + 6 * h + 6, :])


                    # ---------- A2b: v / geo_v (one N=512 pass) ----------
                    for ti in range(16):
                        ps = psA.tile([128, QB], F32, tag="s")
                        for kc in range(8):
                            nc.tensor.matmul(
                                ps[:],
                                xT_sb[:, kc, 128 * ti:128 * (ti + 1)],
                                wvg_sb[:, kc, :],
                                start=(kc == 0), stop=(kc == 7))
                        for ci, (dst, biasb) in enumerate(((vplus, bvb),
                                                           (gvplus, bgvb))):
                            nc.vector.tensor_add(
                                dst[:, ti, :].rearrange(
                                    "p (h c) -> p h c", c=65)[:, :, 0:64],
                                ps[:, 256 * ci:256 * (ci + 1)].rearrange(
                                    "p (h c) -> p h c", c=64),
                                biasb[:].rearrange("p (h c) -> p h c", c=64))


            # ---------- Phase B: dual-path attention ----------
            with (
                tc.tile_pool(name="psU", bufs=2, space=bass.MemorySpace.PSUM) as psU,
                tc.tile_pool(name="psL", bufs=2, space=bass.MemorySpace.PSUM) as psL,
                tc.tile_pool(name="pp", bufs=8) as pp,
                tc.tile_pool(name="cbp", bufs=4) as cbp,
                tc.tile_pool(name="rowp", bufs=4) as rowp,
            ):
                for j in range(NQB):
                    for hp in range(NHP):
                        he, ho = 2 * hp, 2 * hp + 1   # local head indices
                        nkt = 4 * (j + 1)
                        Us = psU.tile([65, 2, QB], F32, tag="u", name="Us")
                        Ug = psU.tile([65, 2, QB], F32, tag="u", name="Ug")
                        for ki, kt in enumerate(range(nkt)):
                            m = kt - 4 * j
                            off = KT * m if m >= 0 else 0
                            lsl = slice(off, QB)
                            qsl = slice(QB * j + off, QB * (j + 1))
                            ksl = slice(KT * kt, KT * (kt + 1))
                            Ls = psL.tile([128, 2, QB], F32, tag="L", name="Ls")
                            Lg = psL.tile([128, 2, QB], F32, tag="L", name="Lg")
                            nc.tensor.matmul(
                                Ls[:, 0, lsl], kTp2[0:64, hp, ksl],
                                qT[0:64, hp, qsl],
                                start=True, stop=True, tile_position=(0, 0))
                            nc.tensor.matmul(
                                Ls[:, 1, lsl], kTp2[64:128, hp, ksl],
                                qT[64:128, hp, qsl],
                                start=True, stop=True, tile_position=(64, 0))
                            nc.tensor.matmul(
                                Lg[:, 0, lsl], jwT[32 * he:32 * he + 6, ksl],
                                rlT[32 * he:32 * he + 6, qsl],
                                start=True, stop=True,
                                tile_position=(32 * he, 0))
                            nc.tensor.matmul(
                                Lg[:, 1, lsl], jwT[32 * ho:32 * ho + 6, ksl],
                                rlT[32 * ho:32 * ho + 6, qsl],
                                start=True, stop=True,
                                tile_position=(32 * ho, 0))
                            Ps = pp.tile([128, 2, QB], BF, tag="P", name="Ps")
                            Pg = pp.tile([128, 2, QB], BF, tag="P", name="Pg")
                            nc.scalar.activation(Ps[:, :, lsl], Ls[:, :, lsl],
                                                 AF.Exp)
                            nc.scalar.activation(Pg[:, :, lsl], Lg[:, :, lsl],
                                                 AF.Exp)
                            if m >= 0:
                                # boundary 128 cols: keep where qc - kr >= 0
                                for Px in (Ps, Pg):
                                    nc.gpsimd.affine_select(
                                        out=Px[:, :, off:off + KT],
                                        in_=Px[:, :, off:off + KT],
                                        compare_op=OP.is_ge, fill=0.0,
                                        base=0, pattern=[[0, 2], [1, KT]],
                                        channel_multiplier=-1)
                            for (U, Pt, vt) in ((Us, Ps, vplus), (Ug, Pg, gvplus)):
                                nc.tensor.matmul(
                                    U[:, 0, lsl], vt[:, kt, 65 * he:65 * he + 65],
                                    Pt[:, 0, lsl],
                                    start=(ki == 0), stop=(ki == nkt - 1))
                                nc.tensor.matmul(
                                    U[:, 1, lsl], vt[:, kt, 65 * ho:65 * ho + 65],
                                    Pt[:, 1, lsl],
                                    start=(ki == 0), stop=(ki == nkt - 1))

                        # ---- combine ----
                        # copy U out of PSUM first so the next block's U MMs
                        # aren't blocked on the (long) normalization chain;
                        # row 64 of the copies is the denominator for free.
                        qsl = slice(QB * j, QB * (j + 1))
                        Usb = cbp.tile([65, 2, QB], F32, tag="ub", name="Usb")
                        Ugb = cbp.tile([65, 2, QB], F32, tag="ub", name="Ugb")
                        nc.vector.tensor_copy(Usb[:], Us[:, :, :])
                        nc.vector.tensor_copy(Ugb[:], Ug[:, :, :])
                        Dc = rowp.tile([4, QB], F32, tag="d")
                        nc.sync.dma_start(Dc[0:2, :], Usb[64:65, :, :])
                        nc.scalar.dma_start(Dc[2:4, :], Ugb[64:65, :, :])
                        rDc = rowp.tile([4, QB], F32, tag="d")
                        nc.vector.reciprocal_approx_fast(out=rDc[:], in_=Dc[:])
                        ac_ = rowp.tile([4, QB], BF, tag="d")
                        nc.vector.tensor_mul(ac_[:], rDc[:], G4[:, qsl])
                        aB = cbp.tile([64, 2, QB], BF, tag="ab", name="aB")
                        bB = cbp.tile([64, 2, QB], BF, tag="ab", name="bB")
                        # SBUF rows can't partition-broadcast directly;
                        # bounce through DRAM scratch (rows r*8 + hp*4 + j)
                        idx = hp * 4 + j
                        nc.sync.dma_start(
                            d_acr[:].rearrange(
                                "(r i) c -> r i c", i=8)[:, idx, :],
                            ac_[:])
                        for r, (dst, sl) in enumerate((
                                (aB, 0), (aB, 1), (bB, 0), (bB, 1))):
                            eng = nc.sync if r % 2 == 0 else nc.scalar
                            eng.dma_start(
                                dst[:, sl, :],
                                bass.AP(tensor=d_acr, offset=(r * 8 + idx) * QB,
                                        ap=[[0, 64], [1, QB]]))
                        u1 = cbp.tile([64, 2, QB], BF, tag="ab", name="u1")
                        u2 = cbp.tile([64, 2, QB], BF, tag="ab", name="u2")
                        nc.vector.tensor_mul(u1[:], Usb[0:64, :, :], aB[:])
                        nc.vector.tensor_mul(u2[:], Ugb[0:64, :, :], bB[:])
                        nc.vector.tensor_add(
                            comb[0:64, hp, qsl], u1[:, 0, :], u2[:, 0, :])
                        codd = cbp.tile([64, QB], BF, tag="co")
                        nc.vector.tensor_add(codd[:], u1[:, 1, :], u2[:, 1, :])
                        nc.sync.dma_start(comb[64:128, hp, qsl], codd[:])

            # ---------- Phase C: final projection ----------
            with (
                tc.tile_pool(name="psC", bufs=4, space=bass.MemorySpace.PSUM) as psC,
                tc.tile_pool(name="outs", bufs=4) as op_,
            ):
                for qt in range(16):
                    for et in range(2):
                        ps = psC.tile([128, QB], F32, tag="s")
                        for hpc in range(2):
                            nc.tensor.matmul(
                                ps[:],
                                comb[:, hpc, 128 * qt:128 * (qt + 1)],
                                outw2_sb[:, hpc, QB * et:QB * (et + 1)],
                                start=(hpc == 0), stop=(hpc == 1))
                        ot = op_.tile([128, QB], BF, tag="o")
                        if et == 0:
                            nc.scalar.copy(ot[:], ps[:])
                        else:
                            nc.vector.tensor_copy(ot[:], ps[:])
                        eng = nc.sync if et == 0 else nc.scalar
                        eng.dma_start(
                            d_partial[128 * qt:128 * (qt + 1),
                                      QB * et:QB * (et + 1)],
                            ot[:])


    nc.compile()
    return nc


_nc_cache = None


def _get_nc():
    global _nc_cache
    if _nc_cache is None:
        _nc_cache = _build_nc()
    return _nc_cache


def _prep_core_inputs(inputs, core):
    b = core // 4
    h0 = (core % 4) * 4
    f = np.float32
    bf = ml_dtypes.bfloat16
    qkv_w, qkv_b = inputs['qkv_w'], inputs['qkv_b']
    scale = DH ** -0.5
    s = slice(h0 * DH, h0 * DH + NH * DH)
    ac = np.ascontiguousarray

    # Operand layout: 64 cols = [write(24)+pad8 | read(24)+pad8]; A/C from w1
    # (shifted = x_prev side), B/D from the w2/read counterparts.
    WLA = np.zeros((D, 64), f); WLB = np.zeros((D, 64), f)
    WLC = np.zeros((D, 64), f); WLD = np.zeros((D, 64), f)
    w1w, w2w = inputs['w1_write'], inputs['w2_write']
    w1r, w2r = inputs['w1_read'], inputs['w2_read']
    for h in range(NH):
        gh = h0 + h
        for jj in range(6):
            i_, j_ = PAIRS4[5 - jj]
            WLA[:, 0 + h * 6 + jj] = w1w[:, gh * 4 + i_] * SIGMA[jj]    # A_w
            WLB[:, 0 + h * 6 + jj] = w2w[:, gh * 4 + j_]                # B_w
            WLC[:, 0 + h * 6 + jj] = w1w[:, gh * 4 + j_] * SIGMA[jj]    # C_w
            WLD[:, 0 + h * 6 + jj] = w2w[:, gh * 4 + i_]                # D_w
        for pp in range(6):
            i_, j_ = PAIRS4[pp]
            WLA[:, 32 + h * 6 + pp] = w1r[:, gh * 4 + i_]               # A_r
            WLB[:, 32 + h * 6 + pp] = w2r[:, gh * 4 + j_]               # B_r
            WLC[:, 32 + h * 6 + pp] = w1r[:, gh * 4 + j_]               # C_r
            WLD[:, 32 + h * 6 + pp] = w2r[:, gh * 4 + i_]               # D_r

    ssel = np.zeros((128, 64), f)
    for half in (0, 32):
        for h in range(NH):
            g = slice(half + 6 * h, half + 6 * h + 6)
            ssel[g, g] = 1.0
    sbc = np.ones((64, 1), f)
    sbc[32:56, 0] = np.repeat(inputs['inc_scale'][h0:h0 + NH], 6).astype(f)

    # out_w rows packed as head pairs: rows 0:64 = even head, 64:128 = odd
    ow = np.asarray(inputs['out_w'], f)
    outw2 = np.zeros((128, 2, D), f)
    for hp in range(2):
        outw2[0:64, hp, :] = ow[(h0 + 2 * hp) * DH:(h0 + 2 * hp + 1) * DH, :]
        outw2[64:128, hp, :] = ow[(h0 + 2 * hp + 1) * DH:(h0 + 2 * hp + 2) * DH, :]

    return {
        'xT': ac(np.asarray(inputs['x'][b], f).T).astype(bf),
        'wq': ac((qkv_w[:, 0 * D:1 * D][:, s] * scale).astype(f)).astype(bf),
        'wk': ac(qkv_w[:, 1 * D:2 * D][:, s].astype(f)).astype(bf),
        'wv': ac(qkv_w[:, 2 * D:3 * D][:, s].astype(f)).astype(bf),
        'wgv': ac(inputs['geo_w'][:, s].astype(f)).astype(bf),
        'wla': np.concatenate(
            [WLA, np.asarray(inputs['gate_w'], f)], axis=1).astype(bf),
        'wlb': WLB.astype(bf),
        'wlc': WLC.astype(bf), 'wld': WLD.astype(bf),
        'outw2': ac(outw2.reshape(128, 2 * D)).astype(bf),
        'bq': ac((qkv_b[0 * D:1 * D][s] * scale).astype(f).reshape(256, 1)),
        'bk': ac(qkv_b[1 * D:2 * D][s].astype(f).reshape(256, 1)),
        'bv': ac(qkv_b[2 * D:3 * D][s].astype(f).reshape(1, 256)).astype(bf),
        'bgv': ac(inputs['geo_b'][s].astype(f).reshape(1, 256)).astype(bf),
        'bgate': ac(inputs['gate_b'].astype(f).reshape(16, 1)),
        'sbc': sbc,
        'ssel': ssel.astype(bf),
        'gsel': np.full((16, 1), 1.0 / 16.0, f),
        'vones': np.ones((128, 64), f).astype(bf),
    }


def kernel(**inputs):
    global LAST_RESULT
    inputs = {k: np.asarray(v) for k, v in inputs.items()}
    nc = _get_nc()
    in_maps = [_prep_core_inputs(inputs, c) for c in range(NCORES)]
    res = run_bass_kernel_spmd(nc, in_maps, core_ids=list(range(NCORES)),
                               trace=TRACE)
    LAST_RESULT = res
    out = np.zeros((B, T, D), np.float32)
    for c in range(NCORES):
        out[c // 4] += np.asarray(res.results[c]['partial'], np.float32)
    out += np.asarray(inputs['out_b'], np.float32)[None, None, :]
    return out


# revision 33
# speedup vs baseline: 1.0611x; 1.0234x over previous
"""DualPathAttention Trainium2 Bass kernel (v2).

Sharding: batch*head parallel across 8 cores. Core c handles batch b=c//4 and
global heads [4*(c%4), 4*(c%4)+4). Each core computes its 4 heads' dual-path
attention and the partial final projection (its 256 rows of out_w); the host
sums the 4 partials per batch (fp32) and adds out_b.

v2 layout/claims (all SBUF operands bf16, PSUM/normalization math fp32):
  - All weights prefetched to SBUF up front (two HWDGE queues), so phase A
    never stalls on DMA.
  - q^T/k^T stored as [128, 2, T]: head pair hp stacked (even head rows 0:64,
    odd 64:128, no zero padding).
  - Phase B per head-pair: std logits via two concurrent K=64 row-tiled MMs
    (tile_position (0,0)/(64,0)); geo logits via two concurrent K=6 MMs at
    32-strips. Diagonal k-tiles narrowed to the live columns ([128m:512]),
    which also shrinks exp and reduces causal masking to one 128-wide
    affine_select over both heads.
  - exp on ACT per (kt, path): [128, 2head, w] PSUM->SBUF bf16; U = [v|1]^T P
    accumulated in PSUM (denominator in row 64 for free).
  - Combine: denominator rows copied p64->p64 (DVE), DMA-relocated to
    partitions 0:4, one reciprocal + gate multiply on [4,512], DMA
    row-broadcast to [64,2,512], two TT muls + adds; odd head's combined
    half DMA-shifted to partitions 64:128 so the final projection runs as
    K=128 head-pair packed MMs.
  - Final projection: 2 MMs per (qt,et) tile, bf16 partial out; host sums.
"""

import os
import numpy as np
import ml_dtypes

import concourse.bass as bass
from concourse import bacc
import concourse.mybir as mybir
import concourse.tile as tile
from concourse.bass_utils import run_bass_kernel_spmd

D, H, B, T = 1024, 16, 2, 2048
DH = 64          # head dim
NH = 4           # heads per core
NHP = 2          # head pairs per core
NCORES = 8
QB = 512         # q block width
KT = 128         # k tile height
NQB = T // QB    # 4
F32 = mybir.dt.float32
BF = mybir.dt.bfloat16

PAIRS4 = [(0, 1), (0, 2), (0, 3), (1, 2), (1, 3), (2, 3)]
SIGMA = [1.0, -1.0, 1.0, 1.0, -1.0, 1.0]

TRACE = False            # set by test harness for profiling runs
LAST_RESULT = None       # BassKernelResults of last run (for exec_time_ns)


def _build_nc():
    nc = bacc.Bacc("TRN2", target_bir_lowering=False, debug=False)

    # ---- DRAM I/O ----
    d_xT = nc.dram_tensor("xT", [D, T], BF, kind="ExternalInput")
    d_wq = nc.dram_tensor("wq", [D, 256], BF, kind="ExternalInput")
    d_wk = nc.dram_tensor("wk", [D, 256], BF, kind="ExternalInput")
    d_wv = nc.dram_tensor("wv", [D, 256], BF, kind="ExternalInput")
    d_wgv = nc.dram_tensor("wgv", [D, 256], BF, kind="ExternalInput")
    d_wla = nc.dram_tensor("wla", [D, 80], BF, kind="ExternalInput")
    d_wlb = nc.dram_tensor("wlb", [D, 64], BF, kind="ExternalInput")
    d_wlc = nc.dram_tensor("wlc", [D, 64], BF, kind="ExternalInput")
    d_wld = nc.dram_tensor("wld", [D, 64], BF, kind="ExternalInput")
    d_outw2 = nc.dram_tensor("outw2", [128, 2 * D], BF, kind="ExternalInput")
    d_bq = nc.dram_tensor("bq", [256, 1], F32, kind="ExternalInput")
    d_bk = nc.dram_tensor("bk", [256, 1], F32, kind="ExternalInput")
    d_bv = nc.dram_tensor("bv", [1, 256], BF, kind="ExternalInput")
    d_bgv = nc.dram_tensor("bgv", [1, 256], BF, kind="ExternalInput")
    d_bgate = nc.dram_tensor("bgate", [16, 1], F32, kind="ExternalInput")
    d_sbc = nc.dram_tensor("sbc", [64, 1], F32, kind="ExternalInput")
    d_ssel = nc.dram_tensor("ssel", [128, 64], BF, kind="ExternalInput")
    d_gsel = nc.dram_tensor("gsel", [16, 1], mybir.dt.float32r, kind="ExternalInput")
    d_vones = nc.dram_tensor("vones", [128, 64], BF, kind="ExternalInput")
    d_partial = nc.dram_tensor("partial", [T, D], BF, kind="ExternalOutput")
    d_acr = nc.dram_tensor("acr", [32, QB], BF, kind="Internal")

    AF = mybir.ActivationFunctionType
    OP = mybir.AluOpType

    with tile.TileContext(nc, linearize=bool(int(os.environ.get('KLIN', '0')))) as tc:
        with (
            tc.tile_pool(name="wC", bufs=1) as wC,
            tc.tile_pool(name="pers", bufs=1) as pers,
            tc.tile_pool(name="wA", bufs=1) as wA,
        ):
            # ---------- weight prefetch ----------
            # lines weights first (phase A1 starts on them), then the fat
            # projection weights; xT itself streams on the gpsimd queue.
            wl_sb = {}
            for i, dw in enumerate((d_wla, d_wlb, d_wlc, d_wld)):
                nc_ = 80 if i == 0 else 64
                t = wA.tile([128, 8, nc_], BF, name=f"wl{i}")
                eng = nc.sync if i % 2 == 0 else nc.scalar
                eng.dma_start(t[:], dw[:].rearrange("(k p) c -> p k c", p=128))
                wl_sb[i] = t
            wq_sb = wA.tile([128, 8, 256], BF)
            wk_sb = wA.tile([128, 8, 256], BF)
            wvg_sb = wA.tile([128, 8, 512], BF)
            nc.sync.dma_start(wq_sb[:], d_wq[:].rearrange("(k p) c -> p k c", p=128))
            nc.scalar.dma_start(wk_sb[:], d_wk[:].rearrange("(k p) c -> p k c", p=128))
            nc.sync.dma_start(wvg_sb[:, :, 0:256],
                              d_wv[:].rearrange("(k p) c -> p k c", p=128))
            nc.scalar.dma_start(wvg_sb[:, :, 256:512],
                                d_wgv[:].rearrange("(k p) c -> p k c", p=128))
            outw2_sb = wC.tile([128, 2, D], BF)
            nc.scalar.dma_start(
                outw2_sb[:], d_outw2[:].rearrange("p (h e) -> p h e", h=2))
            bq_sb = wA.tile([128, 2], F32)
            bk_sb = wA.tile([128, 2], F32)
            nc.sync.dma_start(bq_sb[:], d_bq[:].rearrange("(m p) o -> p (m o)", p=128))
            nc.scalar.dma_start(bk_sb[:], d_bk[:].rearrange("(m p) o -> p (m o)", p=128))
            # bias rows broadcast across partitions for the v/gv TT-add
            bvb = wA.tile([128, 256], BF)
            bgvb = wA.tile([128, 256], BF)
            nc.sync.dma_start(bvb[:], bass.AP(tensor=d_bv, offset=0,
                                              ap=[[0, 128], [1, 256]]))
            nc.scalar.dma_start(bgvb[:], bass.AP(tensor=d_bgv, offset=0,
                                                 ap=[[0, 128], [1, 256]]))
            bgate_sb = wA.tile([80, 1], F32)
            nc.sync.dma_start(bgate_sb[64:80, :], d_bgate[:])
            sbc_sb = wA.tile([64, 1], F32)
            nc.scalar.dma_start(sbc_sb[:], d_sbc[:])
            ssel_sb = wA.tile([128, 64], BF)
            nc.sync.dma_start(ssel_sb[:], d_ssel[:])
            gsel_sb = wA.tile([80, 1], mybir.dt.float32r)
            nc.scalar.dma_start(gsel_sb[64:80, :], d_gsel[:])

            # ---------- persistent B-phase tensors ----------
            qT = pers.tile([128, 2, T], BF)      # pair hp: even 0:64, odd 64:128
            kTp2 = pers.tile([128, 2, T], BF)
            vplus = pers.tile([128, 16, NH * 65], BF)
            gvplus = pers.tile([128, 16, NH * 65], BF)
            jwT = pers.tile([128, T], BF)        # head h lines at [32h, 32h+6)
            rlT = pers.tile([128, T], BF)
            comb = pers.tile([128, 2, T], BF)    # pair hp, even 0:64 odd 64:128
            G4 = pers.tile([4, T], F32)          # rows: 1-g, 1-g, g, g
            g_row = pers.tile([1, T], F32)
            g1m_row = pers.tile([1, T], F32)

            # ones columns of v/gv (col 64 of each head's 65-wide group)
            nc.sync.dma_start(
                vplus[:].rearrange("p t (h c) -> p t h c", c=65)[:, :, :, 64:65],
                d_vones[:].rearrange("p (t h) -> p t h", h=NH))
            nc.scalar.dma_start(
                gvplus[:].rearrange("p t (h c) -> p t h c", c=65)[:, :, :, 64:65],
                d_vones[:].rearrange("p (t h) -> p t h", h=NH))

            with tc.tile_pool(name="xp", bufs=1) as xp:
                xT_sb = xp.tile([128, 8, T], BF)
                for ko in range(8):
                    nc.gpsimd.dma_start(
                        out=xT_sb[:, ko, :], in_=d_xT[128 * ko:128 * (ko + 1), :])

                with tc.tile_pool(name="psA", bufs=6,
                                  space=bass.MemorySpace.PSUM) as psA:
                    # ---------- A1: Pluecker lines ----------
                    with tc.tile_pool(name="lines", bufs=1) as lnp:
                        def _project(dst, wsb, mrows=64):
                            pss = [psA.tile([mrows, QB], F32, tag="s",
                                            name=f"lps{tb}")
                                   for tb in range(NQB)]
                            for kc in range(8):
                                for tb in range(NQB):
                                    nc.tensor.matmul(
                                        pss[tb][:], wsb[:, kc, :],
                                        xT_sb[:, kc, QB * tb:QB * (tb + 1)],
                                        start=(kc == 0), stop=(kc == 7))
                            for tb in range(NQB):
                                nc.scalar.copy(dst[:, QB * tb:QB * (tb + 1)],
                                               pss[tb][:])

                        def _product(t, wa, wb, mx=64):
                            PX = lnp.tile([mx, T], F32, tag="a", name="PX")
                            PY = lnp.tile([64, T], F32, tag="b", name="PY")
                            _project(PX, wa, mrows=mx)
                            _project(PY, wb)
                            # rows 0:32 write-path (uses x_prev on the A side)
                            nc.vector.tensor_mul(
                                t[0:32, 1:T], PX[0:32, 0:T - 1], PY[0:32, 1:T])
                            nc.gpsimd.affine_select(
                                out=t[0:32, 0:1], in_=t[0:32, 1:2],
                                compare_op=OP.is_gt, fill=0.0,
                                base=0, pattern=[[0, 1]], channel_multiplier=0)
                            nc.vector.tensor_mul(
                                t[32:64, :], PX[32:64, :], PY[32:64, :])
                            return PX

                        t1 = lnp.tile([64, T], F32, tag="e")
                        t2 = lnp.tile([64, T], F32, tag="f")
                        t2b = lnp.tile([128, T], BF, tag="g")   # squares, padded
                        nc.vector.memset(t2b[64:128, :], 0.0)
                        PXg = _product(t1, wl_sb[0], wl_sb[1], mx=80)
                        _product(t2, wl_sb[2], wl_sb[3])
                        # gate logits rode along in rows 64:80 of the wla
                        # projection; sigmoid + mean-MM from there
                        for tb in range(NQB):
                            gsig = wA.tile([80, QB], mybir.dt.float32r,
                                           tag="gs", name=f"gsig{tb}")
                            nc.scalar.activation(
                                out=gsig[64:80, :],
                                in_=PXg[64:80, QB * tb:QB * (tb + 1)],
                                func=AF.Sigmoid,
                                bias=bgate_sb[64:80, 0:1], scale=1.0)
                            psg = psA.tile([1, QB], F32, tag="s")
                            nc.tensor.matmul(psg[:], gsel_sb[64:80, :],
                                             gsig[64:80, :],
                                             start=True, stop=True,
                                             tile_position=(64, 0))
                            nc.vector.tensor_copy(
                                g_row[:, QB * tb:QB * (tb + 1)], psg[:])
                        nc.vector.tensor_scalar(
                            out=g1m_row[:], in0=g_row[:],
                            scalar1=-1.0, scalar2=1.0, op0=OP.mult, op1=OP.add)
                        nc.sync.dma_start(G4[0:1, :], g1m_row[:])
                        nc.scalar.dma_start(G4[1:2, :], g1m_row[:])
                        nc.sync.dma_start(G4[2:3, :], g_row[:])
                        nc.scalar.dma_start(G4[3:4, :], g_row[:])
                        nc.vector.tensor_sub(t1[:], t1[:], t2[:])   # lines_u

                        # q/k projections here: their MMs fill the PE while
                        # the DVE/ACT line-normalization tail runs
                        for mc in range(2):
                            for (wsb, bias, dst) in ((wq_sb, bq_sb, qT),
                                                     (wk_sb, bk_sb, kTp2)):
                                for tb in range(NQB):
                                    ps = psA.tile([128, QB], F32, tag="s")
                                    for kc in range(8):
                                        nc.tensor.matmul(
                                            ps[:],
                                            wsb[:, kc, 128 * mc:128 * (mc + 1)],
                                            xT_sb[:, kc, QB * tb:QB * (tb + 1)],
                                            start=(kc == 0), stop=(kc == 7))
                                    nc.vector.tensor_scalar_add(
                                        out=dst[:, mc, QB * tb:QB * (tb + 1)],
                                        in0=ps[:], scalar1=bias[:, mc:mc + 1])

                        nc.scalar.square(t2b[0:64, :], t1[:])       # squares bf16

                        ssq = lnp.tile([64, T], F32, tag="a")
                        for tb in range(NQB):
                            ps = psA.tile([64, QB], F32, tag="s")
                            nc.tensor.matmul(
                                ps[:], ssel_sb[:],
                                t2b[:, QB * tb:QB * (tb + 1)],
                                start=True, stop=True)
                            nc.vector.tensor_scalar_max(
                                out=ssq[:, QB * tb:QB * (tb + 1)], in0=ps[:],
                                scalar1=1e-24)
                        rt = lnp.tile([64, T], F32, tag="b")
                        nc.scalar.sqrt(rt[:], ssq[:])
                        inv = lnp.tile([64, T], F32, tag="a")
                        nc.vector.reciprocal_approx_fast(out=inv[:], in_=rt[:])
                        # fold inc_scale into read-line norms (rows 0:32 are 1.0)
                        nc.vector.tensor_scalar_mul(
                            out=inv[:], in0=inv[:], scalar1=sbc_sb[:, 0:1])
                        lbf = lnp.tile([64, T], BF, tag="f")
                        nc.vector.tensor_mul(lbf[:], t1[:], inv[:])  # lines_n

                        # scatter to 32-aligned per-head layout via DMA
                        for h in range(NH):
                            eng = nc.sync if h % 2 == 0 else nc.scalar
                            eng.dma_start(
                                out=jwT[32 * h:32 * h + 6, :],
                                in_=lbf[6 * h:6 * h + 6, :])
                            eng.dma_start(
                                out=rlT[32 * h:32 * h + 6, :],
                                in_=lbf[32 + 6 * h:32 + 6 * h + 6, :])


                    # ---------- A2b: v / geo_v (one N=512 pass) ----------
                    for ti in range(16):
                        ps = psA.tile([128, QB], F32, tag="s")
                        for kc in range(8):
                            nc.tensor.matmul(
                                ps[:],
                                xT_sb[:, kc, 128 * ti:128 * (ti + 1)],
                                wvg_sb[:, kc, :],
                                start=(kc == 0), stop=(kc == 7))
                        for ci, (dst, biasb) in enumerate(((vplus, bvb),
                                                           (gvplus, bgvb))):
                            nc.vector.tensor_add(
                                dst[:, ti, :].rearrange(
                                    "p (h c) -> p h c", c=65)[:, :, 0:64],
                                ps[:, 256 * ci:256 * (ci + 1)].rearrange(
                                    "p (h c) -> p h c", c=64),
                                biasb[:].rearrange("p (h c) -> p h c", c=64))


            # ---------- Phase B: dual-path attention ----------
            with (
                tc.tile_pool(name="psU", bufs=2, space=bass.MemorySpace.PSUM) as psU,
                tc.tile_pool(name="psL", bufs=2, space=bass.MemorySpace.PSUM) as psL,
                tc.tile_pool(name="pp", bufs=8) as pp,
                tc.tile_pool(name="cbp", bufs=6) as cbp,
                tc.tile_pool(name="rowp", bufs=6) as rowp,
            ):
                for j in range(NQB):
                    for hp in range(NHP):
                        he, ho = 2 * hp, 2 * hp + 1   # local head indices
                        nkt = 4 * (j + 1)
                        Us = psU.tile([65, 2, QB], F32, tag="u", name="Us")
                        Ug = psU.tile([65, 2, QB], F32, tag="u", name="Ug")
                        for ki, kt in enumerate(range(nkt)):
                            m = kt - 4 * j
                            off = KT * m if m >= 0 else 0
                            lsl = slice(off, QB)
                            qsl = slice(QB * j + off, QB * (j + 1))
                            ksl = slice(KT * kt, KT * (kt + 1))
                            Ls = psL.tile([128, 2, QB], F32, tag="L", name="Ls")
                            Lg = psL.tile([128, 2, QB], F32, tag="L", name="Lg")
                            nc.tensor.matmul(
                                Ls[:, 0, lsl], kTp2[0:64, hp, ksl],
                                qT[0:64, hp, qsl],
                                start=True, stop=True, tile_position=(0, 0))
                            nc.tensor.matmul(
                                Ls[:, 1, lsl], kTp2[64:128, hp, ksl],
                                qT[64:128, hp, qsl],
                                start=True, stop=True, tile_position=(64, 0))
                            nc.tensor.matmul(
                                Lg[:, 0, lsl], jwT[32 * he:32 * he + 6, ksl],
                                rlT[32 * he:32 * he + 6, qsl],
                                start=True, stop=True,
                                tile_position=(32 * he, 0))
                            nc.tensor.matmul(
                                Lg[:, 1, lsl], jwT[32 * ho:32 * ho + 6, ksl],
                                rlT[32 * ho:32 * ho + 6, qsl],
                                start=True, stop=True,
                                tile_position=(32 * ho, 0))
                            Ps = pp.tile([128, 2, QB], BF, tag="P", name="Ps")
                            Pg = pp.tile([128, 2, QB], BF, tag="P", name="Pg")
                            nc.scalar.activation(Ps[:, :, lsl], Ls[:, :, lsl],
                                                 AF.Exp)
                            nc.scalar.activation(Pg[:, :, lsl], Lg[:, :, lsl],
                                                 AF.Exp)
                            if m >= 0:
                                # boundary 128 cols: keep where qc - kr >= 0
                                for Px in (Ps, Pg):
                                    nc.gpsimd.affine_select(
                                        out=Px[:, :, off:off + KT],
                                        in_=Px[:, :, off:off + KT],
                                        compare_op=OP.is_ge, fill=0.0,
                                        base=0, pattern=[[0, 2], [1, KT]],
                                        channel_multiplier=-1)
                            for (U, Pt, vt) in ((Us, Ps, vplus), (Ug, Pg, gvplus)):
                                nc.tensor.matmul(
                                    U[:, 0, lsl], vt[:, kt, 65 * he:65 * he + 65],
                                    Pt[:, 0, lsl],
                                    start=(ki == 0), stop=(ki == nkt - 1))
                                nc.tensor.matmul(
                                    U[:, 1, lsl], vt[:, kt, 65 * ho:65 * ho + 65],
                                    Pt[:, 1, lsl],
                                    start=(ki == 0), stop=(ki == nkt - 1))

                        # ---- combine ----
                        # copy U out of PSUM first so the next block's U MMs
                        # aren't blocked on the (long) normalization chain;
                        # row 64 of the copies is the denominator for free.
                        qsl = slice(QB * j, QB * (j + 1))
                        Usb = cbp.tile([65, 2, QB], F32, tag="ub", name="Usb")
                        Ugb = cbp.tile([65, 2, QB], F32, tag="ub", name="Ugb")
                        nc.vector.tensor_copy(Usb[:], Us[:, :, :])
                        nc.vector.tensor_copy(Ugb[:], Ug[:, :, :])
                        Dc = rowp.tile([4, QB], F32, tag="d")
                        nc.sync.dma_start(Dc[0:2, :], Usb[64:65, :, :])
                        nc.scalar.dma_start(Dc[2:4, :], Ugb[64:65, :, :])
                        rDc = rowp.tile([4, QB], F32, tag="d")
                        nc.vector.reciprocal_approx_fast(out=rDc[:], in_=Dc[:])
                        ac_ = rowp.tile([4, QB], BF, tag="d")
                        nc.vector.tensor_mul(ac_[:], rDc[:], G4[:, qsl])
                        aB = cbp.tile([64, 2, QB], BF, tag="ab", name="aB")
                        bB = cbp.tile([64, 2, QB], BF, tag="ab", name="bB")
                        # SBUF rows can't partition-broadcast directly;
                        # bounce through DRAM scratch (rows r*8 + hp*4 + j)
                        idx = hp * 4 + j
                        nc.sync.dma_start(
                            d_acr[:].rearrange(
                                "(r i) c -> r i c", i=8)[:, idx, :],
                            ac_[:])
                        for r, (dst, sl) in enumerate((
                                (aB, 0), (aB, 1), (bB, 0), (bB, 1))):
                            eng = nc.sync if r % 2 == 0 else nc.scalar
                            eng.dma_start(
                                dst[:, sl, :],
                                bass.AP(tensor=d_acr, offset=(r * 8 + idx) * QB,
                                        ap=[[0, 64], [1, QB]]))
                        u1 = cbp.tile([64, 2, QB], BF, tag="ab", name="u1")
                        u2 = cbp.tile([64, 2, QB], BF, tag="ab", name="u2")
                        nc.vector.tensor_mul(u1[:], Usb[0:64, :, :], aB[:])
                        nc.vector.tensor_mul(u2[:], Ugb[0:64, :, :], bB[:])
                        nc.vector.tensor_add(
                            comb[0:64, hp, qsl], u1[:, 0, :], u2[:, 0, :])
                        codd = cbp.tile([64, QB], BF, tag="co")
                        nc.vector.tensor_add(codd[:], u1[:, 1, :], u2[:, 1, :])
                        nc.sync.dma_start(comb[64:128, hp, qsl], codd[:])

            # ---------- Phase C: final projection ----------
            # both 512-wide halves of a q-tile accumulate into one 2-bank
            # PSUM tile so each PSUM->SBUF copy moves 1024 columns: the copy
            # engines (ACT/DVE alternating) are phase C's bottleneck
            with (
                tc.tile_pool(name="psC", bufs=4, space=bass.MemorySpace.PSUM) as psC,
                tc.tile_pool(name="outs", bufs=4) as op_,
            ):
                for qt in range(16):
                    ps = psC.tile([128, 2, QB], F32, tag="s")
                    for et in range(2):
                        for hpc in range(2):
                            nc.tensor.matmul(
                                ps[:, et, :],
                                comb[:, hpc, 128 * qt:128 * (qt + 1)],
                                outw2_sb[:, hpc, QB * et:QB * (et + 1)],
                                start=(hpc == 0), stop=(hpc == 1))
                    ot = op_.tile([128, 2, QB], BF, tag="o")
                    if qt % 2 == 0:
                        nc.scalar.copy(ot[:], ps[:])
                    else:
                        nc.vector.tensor_copy(ot[:], ps[:])
                    eng = nc.sync if qt % 2 == 0 else nc.scalar
                    eng.dma_start(
                        d_partial[128 * qt:128 * (qt + 1), :], ot[:])


    nc.compile()
    return nc


_nc_cache = None


def _get_nc():
    global _nc_cache
    if _nc_cache is None:
        _nc_cache = _build_nc()
    return _nc_cache


def _prep_core_inputs(inputs, core):
    b = core // 4
    h0 = (core % 4) * 4
    f = np.float32
    bf = ml_dtypes.bfloat16
    qkv_w, qkv_b = inputs['qkv_w'], inputs['qkv_b']
    scale = DH ** -0.5
    s = slice(h0 * DH, h0 * DH + NH * DH)
    ac = np.ascontiguousarray

    # Operand layout: 64 cols = [write(24)+pad8 | read(24)+pad8]; A/C from w1
    # (shifted = x_prev side), B/D from the w2/read counterparts.
    WLA = np.zeros((D, 64), f); WLB = np.zeros((D, 64), f)
    WLC = np.zeros((D, 64), f); WLD = np.zeros((D, 64), f)
    w1w, w2w = inputs['w1_write'], inputs['w2_write']
    w1r, w2r = inputs['w1_read'], inputs['w2_read']
    for h in range(NH):
        gh = h0 + h
        for jj in range(6):
            i_, j_ = PAIRS4[5 - jj]
            WLA[:, 0 + h * 6 + jj] = w1w[:, gh * 4 + i_] * SIGMA[jj]    # A_w
            WLB[:, 0 + h * 6 + jj] = w2w[:, gh * 4 + j_]                # B_w
            WLC[:, 0 + h * 6 + jj] = w1w[:, gh * 4 + j_] * SIGMA[jj]    # C_w
            WLD[:, 0 + h * 6 + jj] = w2w[:, gh * 4 + i_]                # D_w
        for pp in range(6):
            i_, j_ = PAIRS4[pp]
            WLA[:, 32 + h * 6 + pp] = w1r[:, gh * 4 + i_]               # A_r
            WLB[:, 32 + h * 6 + pp] = w2r[:, gh * 4 + j_]               # B_r
            WLC[:, 32 + h * 6 + pp] = w1r[:, gh * 4 + j_]               # C_r
            WLD[:, 32 + h * 6 + pp] = w2r[:, gh * 4 + i_]               # D_r

    ssel = np.zeros((128, 64), f)
    for half in (0, 32):
        for h in range(NH):
            g = slice(half + 6 * h, half + 6 * h + 6)
            ssel[g, g] = 1.0
    sbc = np.ones((64, 1), f)
    sbc[32:56, 0] = np.repeat(inputs['inc_scale'][h0:h0 + NH], 6).astype(f)

    # out_w rows packed as head pairs: rows 0:64 = even head, 64:128 = odd
    ow = np.asarray(inputs['out_w'], f)
    outw2 = np.zeros((128, 2, D), f)
    for hp in range(2):
        outw2[0:64, hp, :] = ow[(h0 + 2 * hp) * DH:(h0 + 2 * hp + 1) * DH, :]
        outw2[64:128, hp, :] = ow[(h0 + 2 * hp + 1) * DH:(h0 + 2 * hp + 2) * DH, :]

    return {
        'xT': ac(np.asarray(inputs['x'][b], f).T).astype(bf),
        'wq': ac((qkv_w[:, 0 * D:1 * D][:, s] * scale).astype(f)).astype(bf),
        'wk': ac(qkv_w[:, 1 * D:2 * D][:, s].astype(f)).astype(bf),
        'wv': ac(qkv_w[:, 2 * D:3 * D][:, s].astype(f)).astype(bf),
        'wgv': ac(inputs['geo_w'][:, s].astype(f)).astype(bf),
        'wla': np.concatenate(
            [WLA, np.asarray(inputs['gate_w'], f)], axis=1).astype(bf),
        'wlb': WLB.astype(bf),
        'wlc': WLC.astype(bf), 'wld': WLD.astype(bf),
        'outw2': ac(outw2.reshape(128, 2 * D)).astype(bf),
        'bq': ac((qkv_b[0 * D:1 * D][s] * scale).astype(f).reshape(256, 1)),
        'bk': ac(qkv_b[1 * D:2 * D][s].astype(f).reshape(256, 1)),
        'bv': ac(qkv_b[2 * D:3 * D][s].astype(f).reshape(1, 256)).astype(bf),
        'bgv': ac(inputs['geo_b'][s].astype(f).reshape(1, 256)).astype(bf),
        'bgate': ac(inputs['gate_b'].astype(f).reshape(16, 1)),
        'sbc': sbc,
        'ssel': ssel.astype(bf),
        'gsel': np.full((16, 1), 1.0 / 16.0, f),
        'vones': np.ones((128, 64), f).astype(bf),
    }


def kernel(**inputs):
    global LAST_RESULT
    inputs = {k: np.asarray(v) for k, v in inputs.items()}
    nc = _get_nc()
    in_maps = [_prep_core_inputs(inputs, c) for c in range(NCORES)]
    res = run_bass_kernel_spmd(nc, in_maps, core_ids=list(range(NCORES)),
                               trace=TRACE)
    LAST_RESULT = res
    out = np.zeros((B, T, D), np.float32)
    for c in range(NCORES):
        out[c // 4] += np.asarray(res.results[c]['partial'], np.float32)
    out += np.asarray(inputs['out_b'], np.float32)[None, None, :]
    return out


# revision 38
# speedup vs baseline: 1.0781x; 1.0160x over previous
"""DualPathAttention Trainium2 Bass kernel (v2).

Sharding: batch*head parallel across 8 cores. Core c handles batch b=c//4 and
global heads [4*(c%4), 4*(c%4)+4). Each core computes its 4 heads' dual-path
attention and the partial final projection (its 256 rows of out_w); the host
sums the 4 partials per batch (fp32) and adds out_b.

v2 layout/claims (all SBUF operands bf16, PSUM/normalization math fp32):
  - All weights prefetched to SBUF up front (two HWDGE queues), so phase A
    never stalls on DMA.
  - q^T/k^T stored as [128, 2, T]: head pair hp stacked (even head rows 0:64,
    odd 64:128, no zero padding).
  - Phase B per head-pair: std logits via two concurrent K=64 row-tiled MMs
    (tile_position (0,0)/(64,0)); geo logits via two concurrent K=6 MMs at
    32-strips. Diagonal k-tiles narrowed to the live columns ([128m:512]),
    which also shrinks exp and reduces causal masking to one 128-wide
    affine_select over both heads.
  - exp on ACT per (kt, path): [128, 2head, w] PSUM->SBUF bf16; U = [v|1]^T P
    accumulated in PSUM (denominator in row 64 for free).
  - Combine: denominator rows copied p64->p64 (DVE), DMA-relocated to
    partitions 0:4, one reciprocal + gate multiply on [4,512], DMA
    row-broadcast to [64,2,512], two TT muls + adds; odd head's combined
    half DMA-shifted to partitions 64:128 so the final projection runs as
    K=128 head-pair packed MMs.
  - Final projection: 2 MMs per (qt,et) tile, bf16 partial out; host sums.
"""

import os
import numpy as np
import ml_dtypes

import concourse.bass as bass
from concourse import bacc
import concourse.mybir as mybir
import concourse.tile as tile
from concourse.bass_utils import run_bass_kernel_spmd

D, H, B, T = 1024, 16, 2, 2048
DH = 64          # head dim
NH = 4           # heads per core
NHP = 2          # head pairs per core
NCORES = 8
QB = 512         # q block width
KT = 128         # k tile height
NQB = T // QB    # 4
F32 = mybir.dt.float32
BF = mybir.dt.bfloat16

PAIRS4 = [(0, 1), (0, 2), (0, 3), (1, 2), (1, 3), (2, 3)]
SIGMA = [1.0, -1.0, 1.0, 1.0, -1.0, 1.0]

TRACE = False            # set by test harness for profiling runs
LAST_RESULT = None       # BassKernelResults of last run (for exec_time_ns)


def _build_nc():
    nc = bacc.Bacc("TRN2", target_bir_lowering=False, debug=False)

    # ---- DRAM I/O ----
    d_xT = nc.dram_tensor("xT", [D, T], BF, kind="ExternalInput")
    d_wq = nc.dram_tensor("wq", [D, 256], BF, kind="ExternalInput")
    d_wk = nc.dram_tensor("wk", [D, 256], BF, kind="ExternalInput")
    d_wv = nc.dram_tensor("wv", [D, 256], BF, kind="ExternalInput")
    d_wgv = nc.dram_tensor("wgv", [D, 256], BF, kind="ExternalInput")
    d_wla = nc.dram_tensor("wla", [D, 80], BF, kind="ExternalInput")
    d_wlb = nc.dram_tensor("wlb", [D, 64], BF, kind="ExternalInput")
    d_wlc = nc.dram_tensor("wlc", [D, 64], BF, kind="ExternalInput")
    d_wld = nc.dram_tensor("wld", [D, 64], BF, kind="ExternalInput")
    d_outw2 = nc.dram_tensor("outw2", [128, 2 * D], BF, kind="ExternalInput")
    d_bq = nc.dram_tensor("bq", [256, 1], F32, kind="ExternalInput")
    d_bk = nc.dram_tensor("bk", [256, 1], F32, kind="ExternalInput")
    d_bv = nc.dram_tensor("bv", [1, 256], BF, kind="ExternalInput")
    d_bgv = nc.dram_tensor("bgv", [1, 256], BF, kind="ExternalInput")
    d_bgate = nc.dram_tensor("bgate", [16, 1], F32, kind="ExternalInput")
    d_sbc = nc.dram_tensor("sbc", [64, 1], F32, kind="ExternalInput")
    d_ssel = nc.dram_tensor("ssel", [128, 64], BF, kind="ExternalInput")
    d_gsel = nc.dram_tensor("gsel", [16, 1], mybir.dt.float32r, kind="ExternalInput")
    d_vones = nc.dram_tensor("vones", [128, 64], BF, kind="ExternalInput")
    d_partial = nc.dram_tensor("partial", [T, D], BF, kind="ExternalOutput")
    d_acr = nc.dram_tensor("acr", [32, QB], BF, kind="Internal")

    AF = mybir.ActivationFunctionType
    OP = mybir.AluOpType

    with tile.TileContext(nc, linearize=bool(int(os.environ.get('KLIN', '0')))) as tc:
        with (
            tc.tile_pool(name="wC", bufs=1) as wC,
            tc.tile_pool(name="pers", bufs=1) as pers,
            tc.tile_pool(name="wA", bufs=1) as wA,
        ):
            # ---------- weight prefetch ----------
            # lines weights first (phase A1 starts on them), then the fat
            # projection weights; xT itself streams on the gpsimd queue.
            wl_sb = {}
            for i, dw in enumerate((d_wla, d_wlb, d_wlc, d_wld)):
                nc_ = 80 if i == 0 else 64
                t = wA.tile([128, 8, nc_], BF, name=f"wl{i}")
                eng = nc.sync if i % 2 == 0 else nc.scalar
                eng.dma_start(t[:], dw[:].rearrange("(k p) c -> p k c", p=128))
                wl_sb[i] = t
            wq_sb = wA.tile([128, 8, 256], BF)
            wk_sb = wA.tile([128, 8, 256], BF)
            wvg_sb = wA.tile([128, 8, 512], BF)
            nc.sync.dma_start(wq_sb[:], d_wq[:].rearrange("(k p) c -> p k c", p=128))
            nc.scalar.dma_start(wk_sb[:], d_wk[:].rearrange("(k p) c -> p k c", p=128))
            nc.sync.dma_start(wvg_sb[:, :, 0:256],
                              d_wv[:].rearrange("(k p) c -> p k c", p=128))
            nc.scalar.dma_start(wvg_sb[:, :, 256:512],
                                d_wgv[:].rearrange("(k p) c -> p k c", p=128))
            outw2_sb = wC.tile([128, 2, D], BF)
            nc.scalar.dma_start(
                outw2_sb[:], d_outw2[:].rearrange("p (h e) -> p h e", h=2))
            bq_sb = wA.tile([128, 2], F32)
            bk_sb = wA.tile([128, 2], F32)
            nc.sync.dma_start(bq_sb[:], d_bq[:].rearrange("(m p) o -> p (m o)", p=128))
            nc.scalar.dma_start(bk_sb[:], d_bk[:].rearrange("(m p) o -> p (m o)", p=128))
            # bias rows broadcast across partitions for the v/gv TT-add
            bvb = wA.tile([128, 256], BF)
            bgvb = wA.tile([128, 256], BF)
            nc.sync.dma_start(bvb[:], bass.AP(tensor=d_bv, offset=0,
                                              ap=[[0, 128], [1, 256]]))
            nc.scalar.dma_start(bgvb[:], bass.AP(tensor=d_bgv, offset=0,
                                                 ap=[[0, 128], [1, 256]]))
            bgate_sb = wA.tile([80, 1], F32)
            nc.sync.dma_start(bgate_sb[64:80, :], d_bgate[:])
            sbc_sb = wA.tile([64, 1], F32)
            nc.scalar.dma_start(sbc_sb[:], d_sbc[:])
            ssel_sb = wA.tile([128, 64], BF)
            nc.sync.dma_start(ssel_sb[:], d_ssel[:])
            gsel_sb = wA.tile([80, 1], mybir.dt.float32r)
            nc.scalar.dma_start(gsel_sb[64:80, :], d_gsel[:])

            # ---------- persistent B-phase tensors ----------
            qT = pers.tile([128, 2, T], BF)      # pair hp: even 0:64, odd 64:128
            kTp2 = pers.tile([128, 2, T], BF)
            vplus = pers.tile([128, 16, NH * 65], BF)
            gvplus = pers.tile([128, 16, NH * 65], BF)
            jwT = pers.tile([128, T], BF)        # head h lines at [32h, 32h+6)
            rlT = pers.tile([128, T], BF)
            comb = pers.tile([128, 2, T], BF)    # pair hp, even 0:64 odd 64:128
            G4 = pers.tile([4, T], F32)          # rows: 1-g, 1-g, g, g
            g_row = pers.tile([1, T], F32)
            g1m_row = pers.tile([1, T], F32)

            # ones columns of v/gv (col 64 of each head's 65-wide group)
            nc.sync.dma_start(
                vplus[:].rearrange("p t (h c) -> p t h c", c=65)[:, :, :, 64:65],
                d_vones[:].rearrange("p (t h) -> p t h", h=NH))
            nc.scalar.dma_start(
                gvplus[:].rearrange("p t (h c) -> p t h c", c=65)[:, :, :, 64:65],
                d_vones[:].rearrange("p (t h) -> p t h", h=NH))

            with tc.tile_pool(name="xp", bufs=1) as xp:
                xT_sb = xp.tile([128, 8, T], BF)
                for ko in range(8):
                    nc.gpsimd.dma_start(
                        out=xT_sb[:, ko, :], in_=d_xT[128 * ko:128 * (ko + 1), :])

                with tc.tile_pool(name="psA", bufs=6,
                                  space=bass.MemorySpace.PSUM) as psA:
                    # ---------- A1: Pluecker lines ----------
                    with tc.tile_pool(name="lines", bufs=1) as lnp:
                        def _project(dst, wsb, mrows=64):
                            pss = [psA.tile([mrows, QB], F32, tag="s",
                                            name=f"lps{tb}")
                                   for tb in range(NQB)]
                            for kc in range(8):
                                for tb in range(NQB):
                                    nc.tensor.matmul(
                                        pss[tb][:], wsb[:, kc, :],
                                        xT_sb[:, kc, QB * tb:QB * (tb + 1)],
                                        start=(kc == 0), stop=(kc == 7))
                            for tb in range(NQB):
                                nc.scalar.copy(dst[:, QB * tb:QB * (tb + 1)],
                                               pss[tb][:])

                        def _product(t, wa, wb, mx=64):
                            PX = lnp.tile([mx, T], F32, tag="a", name="PX")
                            PY = lnp.tile([64, T], F32, tag="b", name="PY")
                            _project(PX, wa, mrows=mx)
                            _project(PY, wb)
                            # rows 0:32 write-path (uses x_prev on the A side)
                            nc.vector.tensor_mul(
                                t[0:32, 1:T], PX[0:32, 0:T - 1], PY[0:32, 1:T])
                            nc.gpsimd.affine_select(
                                out=t[0:32, 0:1], in_=t[0:32, 1:2],
                                compare_op=OP.is_gt, fill=0.0,
                                base=0, pattern=[[0, 1]], channel_multiplier=0)
                            nc.vector.tensor_mul(
                                t[32:64, :], PX[32:64, :], PY[32:64, :])
                            return PX

                        t1 = lnp.tile([64, T], F32, tag="e")
                        t2 = lnp.tile([64, T], F32, tag="f")
                        t2b = lnp.tile([128, T], BF, tag="g")   # squares, padded
                        nc.vector.memset(t2b[64:128, :], 0.0)
                        PXg = _product(t1, wl_sb[0], wl_sb[1], mx=80)
                        _product(t2, wl_sb[2], wl_sb[3])
                        # gate logits rode along in rows 64:80 of the wla
                        # projection; sigmoid + mean-MM from there
                        for tb in range(NQB):
                            gsig = wA.tile([80, QB], mybir.dt.float32r,
                                           tag="gs", name=f"gsig{tb}")
                            nc.scalar.activation(
                                out=gsig[64:80, :],
                                in_=PXg[64:80, QB * tb:QB * (tb + 1)],
                                func=AF.Sigmoid,
                                bias=bgate_sb[64:80, 0:1], scale=1.0)
                            psg = psA.tile([1, QB], F32, tag="s")
                            nc.tensor.matmul(psg[:], gsel_sb[64:80, :],
                                             gsig[64:80, :],
                                             start=True, stop=True,
                                             tile_position=(64, 0))
                            nc.vector.tensor_copy(
                                g_row[:, QB * tb:QB * (tb + 1)], psg[:])
                        nc.vector.tensor_scalar(
                            out=g1m_row[:], in0=g_row[:],
                            scalar1=-1.0, scalar2=1.0, op0=OP.mult, op1=OP.add)
                        nc.sync.dma_start(G4[0:1, :], g1m_row[:])
                        nc.scalar.dma_start(G4[1:2, :], g1m_row[:])
                        nc.sync.dma_start(G4[2:3, :], g_row[:])
                        nc.scalar.dma_start(G4[3:4, :], g_row[:])
                        nc.vector.tensor_sub(t1[:], t1[:], t2[:])   # lines_u

                        # q/k projections here: their MMs fill the PE while
                        # the DVE/ACT line-normalization tail runs
                        for mc in range(2):
                            for (wsb, bias, dst) in ((wq_sb, bq_sb, qT),
                                                     (wk_sb, bk_sb, kTp2)):
                                for tb in range(NQB):
                                    ps = psA.tile([128, QB], F32, tag="s")
                                    for kc in range(8):
                                        nc.tensor.matmul(
                                            ps[:],
                                            wsb[:, kc, 128 * mc:128 * (mc + 1)],
                                            xT_sb[:, kc, QB * tb:QB * (tb + 1)],
                                            start=(kc == 0), stop=(kc == 7))
                                    nc.vector.tensor_scalar_add(
                                        out=dst[:, mc, QB * tb:QB * (tb + 1)],
                                        in0=ps[:], scalar1=bias[:, mc:mc + 1])

                        nc.scalar.square(t2b[0:64, :], t1[:])       # squares bf16

                        ssq = lnp.tile([64, T], F32, tag="a")
                        for tb in range(NQB):
                            ps = psA.tile([64, QB], F32, tag="s")
                            nc.tensor.matmul(
                                ps[:], ssel_sb[:],
                                t2b[:, QB * tb:QB * (tb + 1)],
                                start=True, stop=True)
                            nc.vector.tensor_scalar_max(
                                out=ssq[:, QB * tb:QB * (tb + 1)], in0=ps[:],
                                scalar1=1e-24)
                        rt = lnp.tile([64, T], F32, tag="b")
                        nc.scalar.sqrt(rt[:], ssq[:])
                        inv = lnp.tile([64, T], F32, tag="a")
                        nc.vector.reciprocal_approx_fast(out=inv[:], in_=rt[:])
                        # fold inc_scale into read-line norms (rows 0:32 are 1.0)
                        nc.vector.tensor_scalar_mul(
                            out=inv[:], in0=inv[:], scalar1=sbc_sb[:, 0:1])
                        lbf = lnp.tile([64, T], BF, tag="f")
                        nc.vector.tensor_mul(lbf[:], t1[:], inv[:])  # lines_n

                        # scatter to 32-aligned per-head layout via DMA
                        for h in range(NH):
                            eng = nc.sync if h % 2 == 0 else nc.scalar
                            eng.dma_start(
                                out=jwT[32 * h:32 * h + 6, :],
                                in_=lbf[6 * h:6 * h + 6, :])
                            eng.dma_start(
                                out=rlT[32 * h:32 * h + 6, :],
                                in_=lbf[32 + 6 * h:32 + 6 * h + 6, :])


                    # ---------- A2b: v / geo_v (one N=512 pass) ----------
                    for ti in range(16):
                        ps = psA.tile([128, QB], F32, tag="s")
                        for kc in range(8):
                            nc.tensor.matmul(
                                ps[:],
                                xT_sb[:, kc, 128 * ti:128 * (ti + 1)],
                                wvg_sb[:, kc, :],
                                start=(kc == 0), stop=(kc == 7))
                        for ci, (dst, biasb) in enumerate(((vplus, bvb),
                                                           (gvplus, bgvb))):
                            nc.vector.tensor_add(
                                dst[:, ti, :].rearrange(
                                    "p (h c) -> p h c", c=65)[:, :, 0:64],
                                ps[:, 256 * ci:256 * (ci + 1)].rearrange(
                                    "p (h c) -> p h c", c=64),
                                biasb[:].rearrange("p (h c) -> p h c", c=64))


            # ---------- Phase B: dual-path attention ----------
            with (
                tc.tile_pool(name="psU", bufs=2, space=bass.MemorySpace.PSUM) as psU,
                tc.tile_pool(name="psL", bufs=2, space=bass.MemorySpace.PSUM) as psL,
                tc.tile_pool(name="pp", bufs=8) as pp,
                tc.tile_pool(name="cbp", bufs=6) as cbp,
                tc.tile_pool(name="rowp", bufs=6) as rowp,
            ):
                for j in range(NQB):
                    for hp in range(NHP):
                        he, ho = 2 * hp, 2 * hp + 1   # local head indices
                        nkt = 4 * (j + 1)
                        Us = psU.tile([65, 2, QB], F32, tag="u", name="Us")
                        Ug = psU.tile([65, 2, QB], F32, tag="u", name="Ug")
                        def _u_mms(pend):
                            ki_, kt_, lsl_, Ps_, Pg_ = pend
                            for (U, Pt, vt) in ((Us, Ps_, vplus),
                                                (Ug, Pg_, gvplus)):
                                nc.tensor.matmul(
                                    U[:, 0, lsl_],
                                    vt[:, kt_, 65 * he:65 * he + 65],
                                    Pt[:, 0, lsl_],
                                    start=(ki_ == 0), stop=(ki_ == nkt - 1))
                                nc.tensor.matmul(
                                    U[:, 1, lsl_],
                                    vt[:, kt_, 65 * ho:65 * ho + 65],
                                    Pt[:, 1, lsl_],
                                    start=(ki_ == 0), stop=(ki_ == nkt - 1))

                        # software-pipelined: U MMs of tile k are emitted
                        # after the logit MMs of tile k+1, so the mask->U
                        # chain never sits ahead of the next exp's inputs
                        # in the PE queue
                        pend = None
                        for ki, kt in enumerate(range(nkt)):
                            m = kt - 4 * j
                            off = KT * m if m >= 0 else 0
                            lsl = slice(off, QB)
                            qsl = slice(QB * j + off, QB * (j + 1))
                            ksl = slice(KT * kt, KT * (kt + 1))
                            Ls = psL.tile([128, 2, QB], F32, tag="L", name="Ls")
                            Lg = psL.tile([128, 2, QB], F32, tag="L", name="Lg")
                            nc.tensor.matmul(
                                Ls[:, 0, lsl], kTp2[0:64, hp, ksl],
                                qT[0:64, hp, qsl],
                                start=True, stop=True, tile_position=(0, 0))
                            nc.tensor.matmul(
                                Ls[:, 1, lsl], kTp2[64:128, hp, ksl],
                                qT[64:128, hp, qsl],
                                start=True, stop=True, tile_position=(64, 0))
                            nc.tensor.matmul(
                                Lg[:, 0, lsl], jwT[32 * he:32 * he + 6, ksl],
                                rlT[32 * he:32 * he + 6, qsl],
                                start=True, stop=True,
                                tile_position=(32 * he, 0))
                            nc.tensor.matmul(
                                Lg[:, 1, lsl], jwT[32 * ho:32 * ho + 6, ksl],
                                rlT[32 * ho:32 * ho + 6, qsl],
                                start=True, stop=True,
                                tile_position=(32 * ho, 0))
                            if pend is not None:
                                _u_mms(pend)
                            Ps = pp.tile([128, 2, QB], BF, tag="P", name="Ps")
                            Pg = pp.tile([128, 2, QB], BF, tag="P", name="Pg")
                            nc.scalar.activation(Ps[:, :, lsl], Ls[:, :, lsl],
                                                 AF.Exp)
                            nc.scalar.activation(Pg[:, :, lsl], Lg[:, :, lsl],
                                                 AF.Exp)
                            if m >= 0:
                                # boundary 128 cols: keep where qc - kr >= 0
                                for Px in (Ps, Pg):
                                    nc.gpsimd.affine_select(
                                        out=Px[:, :, off:off + KT],
                                        in_=Px[:, :, off:off + KT],
                                        compare_op=OP.is_ge, fill=0.0,
                                        base=0, pattern=[[0, 2], [1, KT]],
                                        channel_multiplier=-1)
                            pend = (ki, kt, lsl, Ps, Pg)
                        _u_mms(pend)

                        # ---- combine ----
                        # copy U out of PSUM first so the next block's U MMs
                        # aren't blocked on the (long) normalization chain;
                        # row 64 of the copies is the denominator for free.
                        qsl = slice(QB * j, QB * (j + 1))
                        Usb = cbp.tile([65, 2, QB], F32, tag="ub", name="Usb")
                        Ugb = cbp.tile([65, 2, QB], F32, tag="ub", name="Ugb")
                        nc.vector.tensor_copy(Usb[:], Us[:, :, :])
                        nc.vector.tensor_copy(Ugb[:], Ug[:, :, :])
                        Dc = rowp.tile([4, QB], F32, tag="d")
                        nc.sync.dma_start(Dc[0:2, :], Usb[64:65, :, :])
                        nc.scalar.dma_start(Dc[2:4, :], Ugb[64:65, :, :])
                        rDc = rowp.tile([4, QB], F32, tag="d")
                        nc.vector.reciprocal_approx_fast(out=rDc[:], in_=Dc[:])
                        ac_ = rowp.tile([4, QB], BF, tag="d")
                        nc.vector.tensor_mul(ac_[:], rDc[:], G4[:, qsl])
                        aB = cbp.tile([64, 2, QB], BF, tag="ab", name="aB")
                        bB = cbp.tile([64, 2, QB], BF, tag="ab", name="bB")
                        # SBUF rows can't partition-broadcast directly;
                        # bounce through DRAM scratch (rows r*8 + hp*4 + j)
                        idx = hp * 4 + j
                        nc.sync.dma_start(
                            d_acr[:].rearrange(
                                "(r i) c -> r i c", i=8)[:, idx, :],
                            ac_[:])
                        for r, (dst, sl) in enumerate((
                                (aB, 0), (aB, 1), (bB, 0), (bB, 1))):
                            eng = nc.sync if r % 2 == 0 else nc.scalar
                            eng.dma_start(
                                dst[:, sl, :],
                                bass.AP(tensor=d_acr, offset=(r * 8 + idx) * QB,
                                        ap=[[0, 64], [1, QB]]))
                        u1 = cbp.tile([64, 2, QB], BF, tag="ab", name="u1")
                        u2 = cbp.tile([64, 2, QB], BF, tag="ab", name="u2")
                        nc.vector.tensor_mul(u1[:], Usb[0:64, :, :], aB[:])
                        nc.vector.tensor_mul(u2[:], Ugb[0:64, :, :], bB[:])
                        nc.vector.tensor_add(
                            comb[0:64, hp, qsl], u1[:, 0, :], u2[:, 0, :])
                        codd = cbp.tile([64, QB], BF, tag="co")
                        nc.vector.tensor_add(codd[:], u1[:, 1, :], u2[:, 1, :])
                        nc.sync.dma_start(comb[64:128, hp, qsl], codd[:])

            # ---------- Phase C: final projection ----------
            # both 512-wide halves of a q-tile accumulate into one 2-bank
            # PSUM tile so each PSUM->SBUF copy moves 1024 columns: the copy
            # engines (ACT/DVE alternating) are phase C's bottleneck
            with (
                tc.tile_pool(name="psC", bufs=4, space=bass.MemorySpace.PSUM) as psC,
                tc.tile_pool(name="outs", bufs=4) as op_,
            ):
                for qt in range(16):
                    ps = psC.tile([128, 2, QB], F32, tag="s")
                    for et in range(2):
                        for hpc in range(2):
                            nc.tensor.matmul(
                                ps[:, et, :],
                                comb[:, hpc, 128 * qt:128 * (qt + 1)],
                                outw2_sb[:, hpc, QB * et:QB * (et + 1)],
                                start=(hpc == 0), stop=(hpc == 1))
                    ot = op_.tile([128, 2, QB], BF, tag="o")
                    if qt % 2 == 0:
                        nc.scalar.copy(ot[:], ps[:])
                    else:
                        nc.vector.tensor_copy(ot[:], ps[:])
                    eng = nc.sync if qt % 2 == 0 else nc.scalar
                    eng.dma_start(
                        d_partial[128 * qt:128 * (qt + 1), :], ot[:])


    nc.compile()
    return nc


_nc_cache = None


def _get_nc():
    global _nc_cache
    if _nc_cache is None:
        _nc_cache = _build_nc()
    return _nc_cache


def _prep_core_inputs(inputs, core):
    b = core // 4
    h0 = (core % 4) * 4
    f = np.float32
    bf = ml_dtypes.bfloat16
    qkv_w, qkv_b = inputs['qkv_w'], inputs['qkv_b']
    scale = DH ** -0.5
    s = slice(h0 * DH, h0 * DH + NH * DH)
    ac = np.ascontiguousarray

    # Operand layout: 64 cols = [write(24)+pad8 | read(24)+pad8]; A/C from w1
    # (shifted = x_prev side), B/D from the w2/read counterparts.
    WLA = np.zeros((D, 64), f); WLB = np.zeros((D, 64), f)
    WLC = np.zeros((D, 64), f); WLD = np.zeros((D, 64), f)
    w1w, w2w = inputs['w1_write'], inputs['w2_write']
    w1r, w2r = inputs['w1_read'], inputs['w2_read']
    for h in range(NH):
        gh = h0 + h
        for jj in range(6):
            i_, j_ = PAIRS4[5 - jj]
            WLA[:, 0 + h * 6 + jj] = w1w[:, gh * 4 + i_] * SIGMA[jj]    # A_w
            WLB[:, 0 + h * 6 + jj] = w2w[:, gh * 4 + j_]                # B_w
            WLC[:, 0 + h * 6 + jj] = w1w[:, gh * 4 + j_] * SIGMA[jj]    # C_w
            WLD[:, 0 + h * 6 + jj] = w2w[:, gh * 4 + i_]                # D_w
        for pp in range(6):
            i_, j_ = PAIRS4[pp]
            WLA[:, 32 + h * 6 + pp] = w1r[:, gh * 4 + i_]               # A_r
            WLB[:, 32 + h * 6 + pp] = w2r[:, gh * 4 + j_]               # B_r
            WLC[:, 32 + h * 6 + pp] = w1r[:, gh * 4 + j_]               # C_r
            WLD[:, 32 + h * 6 + pp] = w2r[:, gh * 4 + i_]               # D_r

    ssel = np.zeros((128, 64), f)
    for half in (0, 32):
        for h in range(NH):
            g = slice(half + 6 * h, half + 6 * h + 6)
            ssel[g, g] = 1.0
    sbc = np.ones((64, 1), f)
    sbc[32:56, 0] = np.repeat(inputs['inc_scale'][h0:h0 + NH], 6).astype(f)

    # out_w rows packed as head pairs: rows 0:64 = even head, 64:128 = odd
    ow = np.asarray(inputs['out_w'], f)
    outw2 = np.zeros((128, 2, D), f)
    for hp in range(2):
        outw2[0:64, hp, :] = ow[(h0 + 2 * hp) * DH:(h0 + 2 * hp + 1) * DH, :]
        outw2[64:128, hp, :] = ow[(h0 + 2 * hp + 1) * DH:(h0 + 2 * hp + 2) * DH, :]

    return {
        'xT': ac(np.asarray(inputs['x'][b], f).T).astype(bf),
        'wq': ac((qkv_w[:, 0 * D:1 * D][:, s] * scale).astype(f)).astype(bf),
        'wk': ac(qkv_w[:, 1 * D:2 * D][:, s].astype(f)).astype(bf),
        'wv': ac(qkv_w[:, 2 * D:3 * D][:, s].astype(f)).astype(bf),
        'wgv': ac(inputs['geo_w'][:, s].astype(f)).astype(bf),
        'wla': np.concatenate(
            [WLA, np.asarray(inputs['gate_w'], f)], axis=1).astype(bf),
        'wlb': WLB.astype(bf),
        'wlc': WLC.astype(bf), 'wld': WLD.astype(bf),
        'outw2': ac(outw2.reshape(128, 2 * D)).astype(bf),
        'bq': ac((qkv_b[0 * D:1 * D][s] * scale).astype(f).reshape(256, 1)),
        'bk': ac(qkv_b[1 * D:2 * D][s].astype(f).reshape(256, 1)),
        'bv': ac(qkv_b[2 * D:3 * D][s].astype(f).reshape(1, 256)).astype(bf),
        'bgv': ac(inputs['geo_b'][s].astype(f).reshape(1, 256)).astype(bf),
        'bgate': ac(inputs['gate_b'].astype(f).reshape(16, 1)),
        'sbc': sbc,
        'ssel': ssel.astype(bf),
        'gsel': np.full((16, 1), 1.0 / 16.0, f),
        'vones': np.ones((128, 64), f).astype(bf),
    }


def kernel(**inputs):
    global LAST_RESULT
    inputs = {k: np.asarray(v) for k, v in inputs.items()}
    nc = _get_nc()
    in_maps = [_prep_core_inputs(inputs, c) for c in range(NCORES)]
    res = run_bass_kernel_spmd(nc, in_maps, core_ids=list(range(NCORES)),
                               trace=TRACE)
    LAST_RESULT = res
    out = np.zeros((B, T, D), np.float32)
    for c in range(NCORES):
        out[c // 4] += np.asarray(res.results[c]['partial'], np.float32)
    out += np.asarray(inputs['out_b'], np.float32)[None, None, :]
    return out
